# revision 26
# baseline (speedup 1.0000x reference)
"""Trainium2 Bass kernel for nn_Experts_66614942761445 (GNN message passing,
K-expert masked GIN). Self-contained: host-side numpy preprocessing + an SPMD
Bass/Tile program on 8 NeuronCores + host-side output assembly.

All phases are node-split: core c owns nodes [6272c, 6272c+6272) (padded to
50176). Segment-sum aggregation runs windowed: for each window of 128
destination slots, dma_gather the source rows (int16 idx, LO/HI split at
32768), build S[e,seg] = (rel[e]==seg) on DVE in fp16, and accumulate
aggT[:,win] += msg.T @ S on PE into f32 PSUM. Node tables are f16 node-major
256B rows (cl tables pack all 4 experts per row -> 512B, one descriptor
serves 4 experts). Tables are rebuilt each layer with an 8-core AllGather.
The edge-mask MLP reuses the same gather streams (transpose-mode gathers give
feature-major Z) and keeps edge weights on-chip, wrapped per window with one
PE transpose. Pooling is an S-matmul against batch membership.
"""
import math
import numpy as np

import concourse.bass as bass
import concourse.bacc as bacc
import concourse.mybir as mybir
import concourse.tile as tile
from concourse.bass_utils import run_bass_kernel_spmd

N, E, H, K, L, B, C = 50000, 800000, 64, 4, 3, 256, 10
P = 128
NCORES = 8
NPC = 6272
WPC = NPC // P            # 49
NPAD = 8 * NPC            # 50176
NW = 8 * WPC              # 392
NLO = 32768
NHI = NPAD - NLO          # 17408

f32 = mybir.dt.float32
f16 = mybir.dt.float16
i16 = mybir.dt.int16
AF = mybir.ActivationFunctionType
OP = mybir.AluOpType

CHUNKS = [(i * 512, 512) for i in range(12)] + [(12 * 512, 128)]


# ----------------------------------------------------------------- host prep
def _wrap_idx(idx):
    n = idx.shape[0]
    t = idx.reshape(n // 16, 16).T
    return np.tile(t, (2, 1)).copy()


def _group_positions(sorted_keys):
    n = sorted_keys.shape[0]
    pos = np.arange(n, dtype=np.int64)
    starts = np.r_[0, np.flatnonzero(np.diff(sorted_keys)) + 1]
    return pos - np.repeat(starts, np.diff(np.r_[starts, n]))


def prep(x, edge_index, batch):
    src = np.asarray(edge_index[0], dtype=np.int64)
    dst = np.asarray(edge_index[1], dtype=np.int64)
    batch = np.asarray(batch, dtype=np.int64)
    x = np.asarray(x, dtype=np.float32)

    w = dst // P
    is_hi = (src >= NLO).astype(np.int64)
    key = w * 2 + is_hi
    order = np.argsort(key, kind="stable")
    jpos = _group_positions(key[order])
    s_src, s_dst, s_w, s_hi = src[order], dst[order], w[order], is_hi[order]

    cnt = np.bincount(key, minlength=2 * NW)
    R_LO = int(math.ceil(cnt[0::2].max() / P))
    R_HI = int(math.ceil(cnt[1::2].max() / P))
    RW = R_LO + R_HI
    SLOTW = RW * P
    TOT = NW * SLOTW
    TOT_PC = WPC * SLOTW

    slots = s_w * SLOTW + np.where(s_hi == 1, R_LO * P + jpos, jpos)
    spread = np.arange(TOT, dtype=np.int64) * 40503
    r_chunk = (np.arange(TOT, dtype=np.int64) // P) % RW
    idx16 = np.where(r_chunk < R_LO, spread % NLO, spread % NHI).astype(np.int16)
    idx16[slots] = np.where(s_hi == 1, s_src - NLO, s_src).astype(np.int16)
    rel = np.full(TOT, -1.0, np.float16)
    rel[slots] = (s_dst - s_w * P).astype(np.float16)
    dsti16 = (spread % NPC).astype(np.int16)
    dsti16[slots] = (s_dst - (s_w // WPC) * NPC).astype(np.int16)
    slot_of_edge = np.empty(E, np.int64)
    slot_of_edge[order] = slots

    gcount = np.bincount(batch, minlength=B).astype(np.float32)
    grecip = (1.0 / np.maximum(gcount, 1.0)).astype(np.float32)
    gbase_core = [int(batch[min(c * NPC, N - 1)]) for c in range(NCORES)]

    def batch_rel_block(lo, gbase, width):
        nb = np.full(NPC, -1.0, np.float32)
        hi = min(lo + NPC, N)
        if hi > lo:
            val = batch[lo:hi] - gbase
            val = np.where((val >= 0) & (val < width), val, -1)
            nb[: hi - lo] = val
        return np.ascontiguousarray(nb.reshape(WPC, P).T.astype(np.float16))

    meta = dict(R_LO=R_LO, R_HI=R_HI, RW=RW, SLOTW=SLOTW, TOT_PC=TOT_PC,
                gbase_core=gbase_core)
    per_core = []
    for c in range(NCORES):
        lo_pc = c * TOT_PC
        d = {
            "gidx": _wrap_idx(idx16[lo_pc:lo_pc + TOT_PC]),
            "grel": np.ascontiguousarray(
                rel[lo_pc:lo_pc + TOT_PC].reshape(-1, P).T),
            "didx": _wrap_idx(dsti16[lo_pc:lo_pc + TOT_PC]),
            "brel_own": batch_rel_block(c * NPC, gbase_core[c], P),
            "brel256": batch_rel_block(c * NPC, 0, 256),
        }
        per_core.append(d)
    return meta, per_core, dict(order=order, slot_of_edge=slot_of_edge,
                                grecip=grecip, gbase_core=gbase_core)


def pack_weights(inp):
    cols, wmap = [], {}

    def addw(name, m):
        m = np.asarray(m, np.float32)
        wmap[name] = (sum(c.shape[1] for c in cols), m.shape[1])
        cols.append(m.astype(np.float16))

    I64 = np.eye(64, dtype=np.float32)
    for l in range(L):
        addw(f"ce_W1_{l}", inp["ce_W1"][l]); addw(f"ce_W2_{l}", inp["ce_W2"][l])
        addw(f"ce_self_{l}", (1.0 + float(inp["ce_eps"][l])) * I64)
        addw(f"cl_W1_{l}", inp["cl_W1"][l]); addw(f"cl_W2_{l}", inp["cl_W2"][l])
        addw(f"cl_self_{l}", (1.0 + float(inp["cl_eps"][l])) * I64)
    for k in range(K):
        addw(f"nm_W1_{k}", inp["nm_W1"][k]); addw(f"nm_W2_{k}", inp["nm_W2"][k])
        addw(f"fm_W1_{k}", inp["fm_W1"][k]); addw(f"fm_W2_{k}", inp["fm_W2"][k])
        addw(f"em_W2_{k}", inp["em_W2"][k]); addw(f"clf_W_{k}", inp["clf_W"][k])
    em_W1 = np.asarray(inp["em_W1"], np.float32)
    for g in range(2):
        k0, k1 = 2 * g, 2 * g + 1
        addw(f"em_W1a_p{g}", np.concatenate([em_W1[k0, :64], em_W1[k1, :64]], 1))
        addw(f"em_W1b_p{g}", np.concatenate([em_W1[k0, 64:], em_W1[k1, 64:]], 1))
    wf16 = np.concatenate(cols, axis=1)

    # [128]-row blockdiag weights in their own pack
    qcols, qmap = [], {}
    for g in range(2):
        k0, k1 = 2 * g, 2 * g + 1
        for nm_, w2 in (("em", inp["em_W2"]), ("nm", inp["nm_W2"])):
            bd = np.zeros((P, 2), np.float32)
            bd[:64, 0] = np.asarray(w2[k0], np.float32).reshape(-1)
            bd[64:, 1] = np.asarray(w2[k1], np.float32).reshape(-1)
            qmap[f"{nm_}_W2bd_p{g}"] = (sum(c.shape[1] for c in qcols), 2)
            qcols.append(bd.astype(np.float16))
    qrows = [0, 1, 32, 33]
    for k in range(K):
        sel = np.zeros((P, 64), np.float32)
        sel[qrows[k], :] = 1.0
        qmap[f"sel_{k}"] = (sum(c.shape[1] for c in qcols), 64)
        qcols.append(sel.astype(np.float16))
    wq16 = np.concatenate(qcols, axis=1)

    bcols, bmap = [], {}

    def addb(name, v):
        col = np.zeros((P, 1), np.float32)
        v = np.asarray(v, np.float32).reshape(-1)
        col[: v.shape[0], 0] = v
        bmap[name] = len(bcols)
        bcols.append(col)

    for l in range(L):
        addb(f"ce_b1_{l}", inp["ce_b1"][l]); addb(f"ce_b2_{l}", inp["ce_b2"][l])
        addb(f"cl_b1_{l}", inp["cl_b1"][l]); addb(f"cl_b2_{l}", inp["cl_b2"][l])
    for k in range(K):
        addb(f"nm_b1_{k}", inp["nm_b1"][k])
        addb(f"fm_b1_{k}", inp["fm_b1"][k]); addb(f"fm_b2_{k}", inp["fm_b2"][k])
        addb(f"clf_b_{k}", inp["clf_b"][k])
    for g in range(2):
        addb(f"em_b1_p{g}", np.r_[np.asarray(inp["em_b1"][2 * g], np.float32),
                                  np.asarray(inp["em_b1"][2 * g + 1], np.float32)])
    for nm_, b2 in (("nm", inp["nm_b2"]), ("em", inp["em_b2"])):
        col = np.zeros(P, np.float32)
        for k in range(K):
            col[(k // 2) * 32 + k % 2] = float(np.asarray(b2[k]).reshape(-1)[0])
        addb(f"{nm_}_b2_q", col)
    bf32 = np.concatenate(bcols, axis=1)
    return wf16, wq16, bf32, wmap, qmap, bmap


# ------------------------------------------------------------- device program
def build_program(meta):
    R_LO, R_HI, RW = meta["R_LO"], meta["R_HI"], meta["RW"]
    SLOTW, TOT_PC = meta["SLOTW"], meta["TOT_PC"]
    gbase_core = meta["gbase_core"]
    wmap, bmap, qmap = meta["wmap"], meta["bmap"], meta["qmap"]
    wc, bc, qc = meta["wc"], meta["bc"], meta["qc"]
    EMROWS = RW * K           # wrapped ew cols per window: (r, k) order

    nc = bacc.Bacc("TRN2", target_bir_lowering=False, debug=False)
    D = nc.dram_tensor

    x_tab = D("x_tab", [NPAD, 2 * H], f16, kind="ExternalInput")
    xT_own = D("xT_own", [H, NPC], f16, kind="ExternalInput")
    gidx_d = D("gidx", [32, TOT_PC // 16], i16, kind="ExternalInput")
    grel_d = D("grel", [P, WPC * RW], f16, kind="ExternalInput")
    didx_d = D("didx", [32, TOT_PC // 16], i16, kind="ExternalInput")
    brelo_d = D("brel_own", [P, WPC], f16, kind="ExternalInput")
    brel2_d = D("brel256", [P, WPC], f16, kind="ExternalInput")
    iota_d = D("iota", [P, P], f16, kind="ExternalInput")
    iota2_d = D("iota256", [P, 256], f16, kind="ExternalInput")
    ident_d = D("ident", [P, P], f16, kind="ExternalInput")
    ones_d = D("ones64", [1, H], f16, kind="ExternalInput")
    wf_d = D("wf16", [H, wc], f16, kind="ExternalInput")
    wq_d = D("wq16", [P, qc], f16, kind="ExternalInput")
    bf_d = D("bf32", [P, bc], f32, kind="ExternalInput")
    grecip_d = D("grecip64", [H, B], f32, kind="ExternalInput")

    nm_out = D("nm_out", [K, NPC], f16, kind="ExternalOutput")
    fm_out = D("fm_out", [K, H, NPC], f16, kind="ExternalOutput")
    ew_out = D("ew_out", [P, WPC * EMROWS], f16, kind="ExternalOutput")
    hs_out = D("hs_out", [K, H, B], f32, kind="ExternalOutput")
    lg_out = D("lg_out", [K, 16, B], f32, kind="ExternalOutput")
    ho_out = D("ho_out", [H, B], f32, kind="ExternalOutput")

    with tile.TileContext(nc) as tc:
        with (
            tc.tile_pool(name="const", bufs=1) as cpool,
            tc.tile_pool(name="strip", bufs=1) as spool,
            tc.tile_pool(name="w1", bufs=1) as wp1,
            tc.tile_pool(name="w2", bufs=2) as wp2,
            tc.tile_pool(name="w3", bufs=2) as wp3,
            tc.tile_pool(name="psagg", bufs=4, space="PSUM") as psagg,
            tc.tile_pool(name="psmlp", bufs=2, space="PSUM") as psmlp,
            tc.tile_pool(name="pstp", bufs=2, space="PSUM") as pstp,
            tc.tile_pool(name="dram", bufs=1, space="DRAM") as dpool,
        ):
            # ---------------- constants
            grel = cpool.tile([P, WPC * RW], f16)
            iota = cpool.tile([P, P], f16)
            iota2 = cpool.tile([P, 256], f16)
            ident = cpool.tile([P, P], f16)
            ones = cpool.tile([1, H], f16)
            wf = cpool.tile([H, wc], f16)
            wq = cpool.tile([P, qc], f16)
            bf = cpool.tile([P, bc], f32)
            brelo = cpool.tile([P, WPC], f16)
            brel2 = cpool.tile([P, WPC], f16)
            grecip = cpool.tile([H, B], f32)
            xT = cpool.tile([H, NPC], f16)
            for t, d in ((grel, grel_d), (iota, iota_d), (iota2, iota2_d),
                         (ident, ident_d), (ones, ones_d), (wf, wf_d),
                         (wq, wq_d), (bf, bf_d), (brelo, brelo_d),
                         (brel2, brel2_d), (grecip, grecip_d), (xT, xT_own)):
                nc.sync.dma_start(t[:], d[:])

            def W(name):
                o, n = wmap[name]
                return wf[:, o:o + n]

            def Wq(name):
                o, n = qmap[name]
                return wq[:, o:o + n]

            def bias(name, p=64):
                return bf[:p, bmap[name]:bmap[name] + 1]

            def bias_q(name):
                # rows {0,1,32,33} as a [2,2,1] AP
                col = bmap[name]
                return bf[:].rearrange("(g q) c -> g q c", q=32)[0:2, 0:2,
                                                                col:col + 1]

            # ---------------- strips & staging
            sA = spool.tile([H, NPC], f16)
            sB = spool.tile([H, NPC], f16)
            aggT = spool.tile([H, NPC], f16)
            cldst = [spool.tile([H, NPC], f16, tag=f"clh{k}", name=f"clh{k}")
                     for k in range(K)]
            stagingP = spool.tile([P, WPC, 4 * H], f16)
            ew = spool.tile([P, WPC, RW, 2, 2], f16)
            nc.gpsimd.memset(stagingP[:, :, H:2 * H], 0.0)

            # ---------------- DRAM tables
            tabs = [dpool.tile([NPAD, 2 * H], f16, tag=f"tab{i}", name=f"tab{i}")
                    for i in range(2)]
            tabZ = dpool.tile([NPAD, 2 * H], f16)
            tabZo = dpool.tile([NPC, 2 * H], f16)
            tabP = [dpool.tile([NPAD, 4 * H], f16, tag=f"tabP{i}", name=f"tabP{i}")
                    for i in range(2)]
            payl = dpool.tile([NPC, 2 * H], f16)
            paylP = dpool.tile([NPC, 4 * H], f16)
            FLAT = K * H * B + H * P
            cc_in = dpool.tile([FLAT], f32)
            cc_out = dpool.tile([NCORES, FLAT], f32)

            def load_idx(w, src_d):
                t = wp2.tile([32, SLOTW // 16], i16, tag="idxw")
                nc.sync.dma_start(t[:], src_d[:, w * (SLOTW // 16):(w + 1) * (SLOTW // 16)])
                return t

            def build_S(w):
                s = wp2.tile([P, RW, P], f16, tag="S")
                nc.vector.tensor_tensor(
                    out=s[:],
                    in0=grel[:, w * RW:(w + 1) * RW, None].to_broadcast([P, RW, P]),
                    in1=iota[:, None, :].to_broadcast([P, RW, P]),
                    op=OP.is_equal)
                return s

            def mlp(dst, src_strip, w1, b1, w2, b2):
                for (o, n) in CHUNKS:
                    p1 = psmlp.tile([P, 512], f32, tag="mlp")
                    nc.tensor.matmul(p1[:H, :n], w1, src_strip[:, o:o + n],
                                     start=True, stop=True)
                    h1 = wp3.tile([H, 512], f16, tag="mlph")
                    nc.scalar.activation(h1[:, :n], p1[:H, :n], AF.Relu, bias=b1)
                    p2 = psmlp.tile([P, 512], f32, tag="mlp")
                    nc.tensor.matmul(p2[:H, :n], w2, h1[:, :n],
                                     start=True, stop=True)
                    nc.scalar.activation(dst[:, o:o + n], p2[:H, :n], AF.Relu,
                                         bias=b2)

            def transpose_to(dst_ap, src_ap, in_parts=H):
                pt = pstp.tile([P, P], f16, tag="tp")
                w_ = src_ap.shape[-1]
                nc.tensor.transpose(out=pt[:w_, :in_parts], in_=src_ap,
                                    identity=ident[:in_parts, :in_parts])
                nc.vector.tensor_copy(dst_ap, pt[:w_, :in_parts])

            def aggregate(table, packed, src_strip, selfW, dst_strip, use_ew=False, k=0):
                elem = 4 * H if packed else 2 * H
                for w in range(WPC):
                    gi = load_idx(w, gidx_d)
                    m = wp2.tile([P, RW, elem], f16,
                                 tag="msg")
                    nc.gpsimd.dma_gather(
                        m[:, :R_LO, :], table[:NLO], gi[:, :R_LO * 8],
                        R_LO * P, R_LO * P, elem, single_packet=False)
                    nc.gpsimd.dma_gather(
                        m[:, R_LO:, :], table[NLO:], gi[:, R_LO * 8:],
                        R_HI * P, R_HI * P, elem, single_packet=False)
                    if use_ew:
                        nc.vector.tensor_tensor(
                            out=m[:, :, k * H:(k + 1) * H],
                            in0=m[:, :, k * H:(k + 1) * H],
                            in1=ew[:, w, :, k // 2, k % 2, None]
                                .to_broadcast([P, RW, H]),
                            op=OP.mult)
                    s = build_S(w)
                    pa = psagg.tile([H, P], f32, tag="agg")
                    for r in range(RW):
                        nc.tensor.matmul(pa[:], m[:, r, k * H:k * H + H],
                                         s[:, r, :], start=(r == 0), stop=False)
                    nc.tensor.matmul(pa[:], selfW,
                                     src_strip[:, w * P:(w + 1) * P],
                                     start=False, stop=True)
                    nc.scalar.activation(dst_strip[:, w * P:(w + 1) * P], pa[:],
                                         AF.Copy)

            def ag8(in_t, out_t):
                nc.gpsimd.collective_compute(
                    "AllGather", OP.bypass,
                    replica_groups=[list(range(NCORES))],
                    ins=[in_t[:]], outs=[out_t[:]])

            # =================== ce GIN ===================
            strips = [xT, sA, sB, sA]
            for l in range(L):
                table = x_tab if l == 0 else tabs[(l + 1) % 2]
                aggregate(table, False, strips[l], W(f"ce_self_{l}"), aggT)
                mlp(strips[l + 1], aggT, W(f"ce_W1_{l}"), bias(f"ce_b1_{l}"),
                    W(f"ce_W2_{l}"), bias(f"ce_b2_{l}"))
                for w in range(WPC):
                    transpose_to(stagingP[:, w, :H],
                                 strips[l + 1][:, w * P:(w + 1) * P])
                nc.sync.dma_start(
                    payl[:].rearrange("(w p) e -> p w e", p=P),
                    stagingP[:, :, :2 * H])
                ag8(payl, tabZ if l == L - 1 else tabs[l % 2])
                if l == L - 1:
                    nc.sync.dma_start(tabZo[:], payl[:])
            ZT = strips[L]   # = sA

            # =================== h_orig partial ===================
            pho = psagg.tile([H, P], f32, tag="agg")
            for w in range(WPC):
                sb = wp2.tile([P, P], f16, tag="sbw")
                nc.vector.tensor_tensor(
                    out=sb[:], in0=brelo[:, w:w + 1].to_broadcast([P, P]),
                    in1=iota[:], op=OP.is_equal)
                nc.tensor.matmul(pho[:], stagingP[:, w, :H], sb[:],
                                 start=(w == 0), stop=(w == WPC - 1))
            ho_part = spool.tile([H, P], f32)
            nc.vector.tensor_copy(ho_part[:], pho[:])

            # =================== masks + masked_x ===================
            for (o, n) in CHUNKS:
                pnm = pstp.tile([P, 512], f32, tag="tp")
                for g in range(2):
                    k0, k1 = 2 * g, 2 * g + 1
                    p1 = psmlp.tile([P, 512], f32, tag="mlp")
                    nc.tensor.matmul(p1[:H, :n], W(f"nm_W1_{k0}"),
                                     ZT[:, o:o + n], start=True, stop=True)
                    nc.tensor.matmul(p1[H:, :n], W(f"nm_W1_{k1}"),
                                     ZT[:, o:o + n], start=True, stop=True)
                    h1 = wp3.tile([P, 512], f16, tag="mlph")
                    nc.scalar.activation(h1[:H, :n], p1[:H, :n], AF.Relu,
                                         bias=bias(f"nm_b1_{k0}"))
                    nc.scalar.activation(h1[H:, :n], p1[H:, :n], AF.Relu,
                                         bias=bias(f"nm_b1_{k1}"))
                    nc.tensor.matmul(pnm[32 * g:32 * g + 2, :n],
                                     Wq(f"nm_W2bd_p{g}"), h1[:, :n],
                                     start=True, stop=True)
                nmc = wp2.tile([34, 512], f16, tag="nmc")
                nc.scalar.activation(nmc[:, :n], pnm[:34, :n], AF.Sigmoid,
                                     bias=bias("nm_b2_q", 34))
                nc.sync.dma_start(nm_out[0:2, o:o + n], nmc[0:2, :n])
                nc.sync.dma_start(nm_out[2:4, o:o + n], nmc[32:34, :n])
                for k in range(K):
                    p2 = psmlp.tile([P, 512], f32, tag="mlp")
                    nc.tensor.matmul(p2[:H, :n], W(f"fm_W1_{k}"), ZT[:, o:o + n],
                                     start=True, stop=True)
                    h2 = wp3.tile([P, 512], f16, tag="mlph")
                    nc.scalar.activation(h2[:H, :n], p2[:H, :n], AF.Relu,
                                         bias=bias(f"fm_b1_{k}"))
                    p3 = psmlp.tile([P, 512], f32, tag="mlp")
                    nc.tensor.matmul(p3[:H, :n], W(f"fm_W2_{k}"), h2[:H, :n],
                                     start=True, stop=True)
                    fk = wp3.tile([P, 512], f16, tag="mlph")
                    nc.scalar.activation(fk[:H, :n], p3[:H, :n], AF.Sigmoid,
                                         bias=bias(f"fm_b2_{k}"))
                    nc.sync.dma_start(fm_out[k, :, o:o + n], fk[:H, :n])
                    pb = psmlp.tile([P, 512], f32, tag="mlp")
                    nc.tensor.matmul(pb[:H, :n], Wq(f"sel_{k}")[:34, :],
                                     nmc[:, :n], start=True, stop=True)
                    nmb = wp3.tile([P, 512], f16, tag="mlph")
                    nc.vector.tensor_copy(nmb[:H, :n], pb[:H, :n])
                    nc.vector.tensor_tensor(out=cldst[k][:, o:o + n],
                                            in0=xT[:, o:o + n],
                                            in1=fk[:H, :n], op=OP.mult)
                    nc.vector.tensor_tensor(out=cldst[k][:, o:o + n],
                                            in0=cldst[k][:, o:o + n],
                                            in1=nmb[:H, :n], op=OP.mult)
                for j in range(n // P):
                    w = (o + j * P) // P
                    for k in range(K):
                        transpose_to(stagingP[:, w, k * H:(k + 1) * H],
                                     cldst[k][:, w * P:(w + 1) * P])
            nc.sync.dma_start(
                paylP[:].rearrange("(w p) e -> p w e", p=P), stagingP[:])
            ag8(paylP, tabP[0])

            # =================== edge masks ===================
            # chunk plan over one window's slots: (src tile key, local off,
            # global off, len)
            emplan = []
            for ch in range(0, R_LO * P, 512):
                emplan.append(("L", ch, ch, min(512, R_LO * P - ch)))
            for ch in range(0, R_HI * P, 512):
                emplan.append(("Hh", ch, R_LO * P + ch, min(512, R_HI * P - ch)))
            for w in range(WPC):
                gi = load_idx(w, gidx_d)
                di = load_idx(w, didx_d)
                zsL = wp1.tile([P, 1, R_LO * P], f16, tag="zsL")
                zsH = wp1.tile([P, 1, R_HI * P], f16, tag="zsH")
                zd = wp1.tile([P, 1, SLOTW], f16, tag="zd")
                nc.gpsimd.dma_gather(zsL[:], tabZ[:NLO], gi[:, :R_LO * 8],
                                     R_LO * P, R_LO * P, 2 * H, transpose=True,
                                     single_packet=False)
                nc.gpsimd.dma_gather(zsH[:], tabZ[NLO:], gi[:, R_LO * 8:],
                                     R_HI * P, R_HI * P, 2 * H, transpose=True,
                                     single_packet=False)
                nc.gpsimd.dma_gather(zd[:], tabZo[:], di[:],
                                     SLOTW, SLOTW, 2 * H, transpose=True,
                                     single_packet=False)
                eo = wp1.tile([34, SLOTW], f16, tag="ewsig")
                for (skey, lo, go, n) in emplan:
                    zs = zsL if skey == "L" else zsH
                    po = psagg.tile([P, 512], f32, tag="agg")
                    for g in range(2):
                        ph2 = psmlp.tile([P, 512], f32, tag="mlp")
                        nc.tensor.matmul(ph2[:, :n], W(f"em_W1a_p{g}"),
                                         zs[:H, 0, lo:lo + n], start=True,
                                         stop=False)
                        nc.tensor.matmul(ph2[:, :n], W(f"em_W1b_p{g}"),
                                         zd[:H, 0, go:go + n], start=False,
                                         stop=True)
                        hh = wp3.tile([P, 512], f16, tag="mlph")
                        if g == 0:
                            nc.scalar.activation(hh[:, :n], ph2[:, :n], AF.Relu,
                                                 bias=bias("em_b1_p0", P))
                        else:
                            nc.vector.tensor_scalar(
                                out=hh[:, :n], in0=ph2[:, :n],
                                scalar1=bias("em_b1_p1", P), scalar2=0.0,
                                op0=OP.add, op1=OP.max)
                        nc.tensor.matmul(po[32 * g:32 * g + 2, :n],
                                         Wq(f"em_W2bd_p{g}"), hh[:, :n],
                                         start=True, stop=True)
                    nc.scalar.activation(eo[:, go:go + n], po[:34, :n],
                                         AF.Sigmoid, bias=bias("em_b2_q", 34))
                # wrap [34, SLOTW] -> ew[:, w, r, g, q] via RW transposes
                pt = pstp.tile([P, RW * 34], f16, tag="tp")
                for r in range(RW):
                    nc.tensor.transpose(out=pt[:, 34 * r:34 * r + 34],
                                        in_=eo[:, r * P:(r + 1) * P],
                                        identity=ident[:34, :34])
                ptv = pt[:].rearrange("p (r e) -> p r e", r=RW)
                nc.vector.tensor_copy(ew[:, w, :, 0, :], ptv[:, :, 0:2])
                nc.vector.tensor_copy(ew[:, w, :, 1, :], ptv[:, :, 32:34])
            nc.sync.dma_start(ew_out[:], ew[:].rearrange("p w r g q -> p (w r g q)"))

            # =================== cl GIN ===================
            for l in range(L):
                table = tabP[l % 2]
                for k in range(K):
                    aggregate(table, True, cldst[k], W(f"cl_self_{l}"), aggT,
                              use_ew=True, k=k)
                    mlp(cldst[k], aggT, W(f"cl_W1_{l}"), bias(f"cl_b1_{l}"),
                        W(f"cl_W2_{l}"), bias(f"cl_b2_{l}"))
                    for w in range(WPC):
                        transpose_to(stagingP[:, w, k * H:(k + 1) * H],
                                     cldst[k][:, w * P:(w + 1) * P])
                if l < L - 1:
                    nc.sync.dma_start(
                        paylP[:].rearrange("(w p) e -> p w e", p=P), stagingP[:])
                    ag8(paylP, tabP[(l + 1) % 2])

            # =================== pooling, h_stable, logits, h_orig ========
            pk = [psagg.tile([H, 256], f32, tag="agg", name=f"pk{k}") for k in range(K)]
            for w in range(WPC):
                sb2 = wp2.tile([P, 256], f16, tag="sb2")
                nc.vector.tensor_tensor(
                    out=sb2[:], in0=brel2[:, w:w + 1].to_broadcast([P, 256]),
                    in1=iota2[:], op=OP.is_equal)
                for k in range(K):
                    nc.tensor.matmul(pk[k][:], stagingP[:, w, k * H:(k + 1) * H],
                                     sb2[:], start=(w == 0), stop=(w == WPC - 1))
            for k in range(K):
                pp = wp2.tile([H, 256], f32, tag="poolp")
                nc.vector.tensor_copy(pp[:], pk[k][:])
                nc.sync.dma_start(
                    cc_in[k * H * B:(k + 1) * H * B]
                    .rearrange("(p e) -> p e", p=H), pp[:])
            nc.sync.dma_start(
                cc_in[K * H * B:].rearrange("(p e) -> p e", p=H), ho_part[:])
            ag8(cc_in, cc_out)

            # combine pools
            for k in range(K):
                acc = wp2.tile([H, 256], f32, tag="acc")
                tmp = wp1.tile([H, NCORES, 256], f32, tag="ptmp")
                nc.sync.dma_start(
                    tmp[:],
                    cc_out[:, k * H * B:(k + 1) * H * B]
                    .rearrange("c (p e) -> p c e", p=H))
                nc.vector.tensor_tensor(out=acc[:], in0=tmp[:, 0, :],
                                        in1=tmp[:, 1, :], op=OP.add)
                for j in range(2, NCORES):
                    nc.vector.tensor_tensor(out=acc[:], in0=acc[:],
                                            in1=tmp[:, j, :], op=OP.add)
                nc.vector.tensor_tensor(out=acc[:], in0=acc[:], in1=grecip[:],
                                        op=OP.mult)
                nc.sync.dma_start(hs_out[k], acc[:])
                h16 = wp1.tile([H, 256], f16, tag="h16")
                nc.vector.tensor_copy(h16[:], acc[:])
                pl = pstp.tile([16, 256], f32, tag="tp")
                nc.tensor.matmul(pl[:C, :], W(f"clf_W_{k}"), h16[:],
                                 start=True, stop=True)
                lgt = wp1.tile([16, 256], f32, tag="lgt")
                nc.vector.tensor_scalar(out=lgt[:C, :], in0=pl[:C, :],
                                        scalar1=bias(f"clf_b_{k}", C),
                                        scalar2=None, op0=OP.add)
                nc.sync.dma_start(lg_out[k, :C, :], lgt[:C, :])

            # combine h_orig
            hof = wp2.tile([H, 256], f32, tag="acc")
            nc.gpsimd.memset(hof[:], 0.0)
            hot = wp1.tile([H, NCORES, P], f32, tag="hot")
            nc.sync.dma_start(
                hot[:], cc_out[:, K * H * B:].rearrange("c (p e) -> p c e", p=H))
            for j in range(NCORES):
                g0 = gbase_core[j]
                nwid = min(P, B - g0)
                nc.vector.tensor_tensor(
                    out=hof[:, g0:g0 + nwid], in0=hof[:, g0:g0 + nwid],
                    in1=hot[:, j, :nwid], op=OP.add)
            nc.vector.tensor_tensor(out=hof[:], in0=hof[:], in1=grecip[:],
                                    op=OP.mult)
            nc.sync.dma_start(ho_out[:], hof[:])

    nc.compile()
    return nc


# ---------------------------------------------------------------- entry point
def make_in_maps(inputs, meta, per_core, post, wf16, wq16, bf32):
    x = np.asarray(inputs["x"], np.float32)
    xpad = np.zeros((NPAD, 2 * H), np.float16)
    xpad[:N, :H] = x.astype(np.float16)
    iota = np.tile(np.arange(P, dtype=np.float16), (P, 1))
    iota2 = np.tile(np.arange(256, dtype=np.float16), (P, 1))
    ident = np.eye(P, dtype=np.float16)
    ones64 = np.ones((1, H), np.float16)
    grecip64 = np.tile(post["grecip"].reshape(1, B), (H, 1)).astype(np.float32)

    in_maps = []
    for c in range(NCORES):
        d = per_core[c]
        xsl = np.zeros((H, NPC), np.float16)
        hi = min((c + 1) * NPC, N)
        xsl[:, : hi - c * NPC] = x[c * NPC:hi].T.astype(np.float16)
        in_maps.append({
            "x_tab": xpad, "xT_own": xsl,
            "gidx": d["gidx"], "grel": d["grel"], "didx": d["didx"],
            "brel_own": d["brel_own"], "brel256": d["brel256"],
            "iota": iota, "iota256": iota2, "ident": ident, "ones64": ones64,
            "wf16": wf16, "wq16": wq16, "bf32": bf32, "grecip64": grecip64,
        })
    return in_maps


def kernel(**inputs):
    x = np.asarray(inputs["x"], np.float32)
    meta, per_core, post = prep(x, inputs["edge_index"], inputs["batch"])
    wf16, wq16, bf32, wmap, qmap, bmap = pack_weights(inputs)
    meta["wmap"], meta["bmap"], meta["qmap"] = wmap, bmap, qmap
    meta["wc"], meta["bc"], meta["qc"] = (wf16.shape[1], bf32.shape[1],
                                          wq16.shape[1])
    nc = build_program(meta)
    in_maps = make_in_maps(inputs, meta, per_core, post, wf16, wq16, bf32)
    res = run_bass_kernel_spmd(nc, in_maps, core_ids=list(range(NCORES)))
    return assemble(res.results, meta, post)


def assemble(results, meta, post):
    RW = meta["RW"]
    # node-split outputs
    nm = np.concatenate([results[c]["nm_out"] for c in range(NCORES)], axis=1)
    fm = np.concatenate([results[c]["fm_out"] for c in range(NCORES)], axis=2)
    node_masks = nm.astype(np.float32).T[:N, :, None]            # [N, K, 1]
    feat_masks = fm.astype(np.float32).transpose(2, 0, 1)[:N]    # [N, K, F]

    # edge masks: ew_out [128, WPC*EMROWS]; value for (p, w, k, r) is slot
    # w*SLOTW + r*128 + p of that core.
    EMR = K * RW
    ews = np.stack([results[c]["ew_out"] for c in range(NCORES)])  # [8,128,WPC*EMR]
    ews = ews.reshape(NCORES, P, WPC, RW, K)
    soe = post["slot_of_edge"]
    c_e = soe // meta["TOT_PC"]
    rem = soe % meta["TOT_PC"]
    w_e = rem // meta["SLOTW"]
    r_e = (rem % meta["SLOTW"]) // P
    p_e = rem % P
    edge_masks = np.empty((E, K, 1), np.float32)
    for k in range(K):
        edge_masks[:, k, 0] = ews[c_e, p_e, w_e, r_e, k]

    hs = results[0]["hs_out"]          # [K, H, B]
    lg = results[0]["lg_out"][:, :C]   # [K, C, B]
    ho = results[0]["ho_out"]          # [H, B]
    expert_logits = lg.transpose(2, 0, 1).astype(np.float32)
    h_stable = hs.transpose(2, 0, 1).astype(np.float32)
    h_orig = ho.T.astype(np.float32)
    return (expert_logits, h_stable, h_orig, node_masks, edge_masks, feat_masks)


# revision 27
# speedup vs baseline: 1.8566x; 1.8566x over previous
"""Trainium2 Bass kernel for nn_Experts_66614942761445 (GNN message passing,
K-expert masked GIN). Self-contained: host-side numpy preprocessing + an SPMD
Bass/Tile program on 8 NeuronCores + host-side output assembly.

All phases are node-split: core c owns nodes [6272c, 6272c+6272) (padded to
50176). Segment-sum aggregation runs windowed: for each window of 128
destination slots, dma_gather the source rows (int16 idx, LO/HI split at
32768), build S[e,seg] = (rel[e]==seg) on DVE in fp16, and accumulate
aggT[:,win] += msg.T @ S on PE into f32 PSUM. Node tables are f16 node-major
256B rows (cl tables pack all 4 experts per row -> 512B, one descriptor
serves 4 experts). Tables are rebuilt each layer with an 8-core AllGather.
The edge-mask MLP reuses the same gather streams (transpose-mode gathers give
feature-major Z) and keeps edge weights on-chip, wrapped per window with one
PE transpose. Pooling is an S-matmul against batch membership.
"""
import math
import numpy as np

import concourse.bass as bass
import concourse.bacc as bacc
import concourse.mybir as mybir
import concourse.tile as tile
from concourse.bass_utils import run_bass_kernel_spmd

N, E, H, K, L, B, C = 50000, 800000, 64, 4, 3, 256, 10
P = 128
NCORES = 8
NPC = 6272
WPC = NPC // P            # 49
NPAD = 8 * NPC            # 50176
NW = 8 * WPC              # 392
NLO = 32768
NHI = NPAD - NLO          # 17408

f32 = mybir.dt.float32
f16 = mybir.dt.float16
i16 = mybir.dt.int16
AF = mybir.ActivationFunctionType
OP = mybir.AluOpType

CHUNKS = [(i * 512, 512) for i in range(12)] + [(12 * 512, 128)]


# ----------------------------------------------------------------- host prep
def _wrap_idx(idx):
    n = idx.shape[0]
    t = idx.reshape(n // 16, 16).T
    return np.tile(t, (2, 1)).copy()


def _group_positions(sorted_keys):
    n = sorted_keys.shape[0]
    pos = np.arange(n, dtype=np.int64)
    starts = np.r_[0, np.flatnonzero(np.diff(sorted_keys)) + 1]
    return pos - np.repeat(starts, np.diff(np.r_[starts, n]))


def prep(x, edge_index, batch):
    src = np.asarray(edge_index[0], dtype=np.int64)
    dst = np.asarray(edge_index[1], dtype=np.int64)
    batch = np.asarray(batch, dtype=np.int64)
    x = np.asarray(x, dtype=np.float32)

    w = dst // P
    is_hi = (src >= NLO).astype(np.int64)
    key = w * 2 + is_hi
    order = np.argsort(key, kind="stable")
    jpos = _group_positions(key[order])
    s_src, s_dst, s_w, s_hi = src[order], dst[order], w[order], is_hi[order]

    cnt = np.bincount(key, minlength=2 * NW)
    R_LO = int(math.ceil(cnt[0::2].max() / P))
    R_HI = int(math.ceil(cnt[1::2].max() / P))
    RW = R_LO + R_HI
    SLOTW = RW * P
    TOT = NW * SLOTW
    TOT_PC = WPC * SLOTW

    slots = s_w * SLOTW + np.where(s_hi == 1, R_LO * P + jpos, jpos)
    spread = np.arange(TOT, dtype=np.int64) * 40503
    r_chunk = (np.arange(TOT, dtype=np.int64) // P) % RW
    idx16 = np.where(r_chunk < R_LO, spread % NLO, spread % NHI).astype(np.int16)
    idx16[slots] = np.where(s_hi == 1, s_src - NLO, s_src).astype(np.int16)
    rel = np.full(TOT, -1.0, np.float16)
    rel[slots] = (s_dst - s_w * P).astype(np.float16)
    dsti16 = (spread % NPC).astype(np.int16)
    dsti16[slots] = (s_dst - (s_w // WPC) * NPC).astype(np.int16)
    slot_of_edge = np.empty(E, np.int64)
    slot_of_edge[order] = slots

    gcount = np.bincount(batch, minlength=B).astype(np.float32)
    grecip = (1.0 / np.maximum(gcount, 1.0)).astype(np.float32)
    gbase_core = [int(batch[min(c * NPC, N - 1)]) for c in range(NCORES)]

    def batch_rel_block(lo, gbase, width):
        nb = np.full(NPC, -1.0, np.float32)
        hi = min(lo + NPC, N)
        if hi > lo:
            val = batch[lo:hi] - gbase
            val = np.where((val >= 0) & (val < width), val, -1)
            nb[: hi - lo] = val
        return np.ascontiguousarray(nb.reshape(WPC, P).T.astype(np.float16))

    meta = dict(R_LO=R_LO, R_HI=R_HI, RW=RW, SLOTW=SLOTW, TOT_PC=TOT_PC,
                gbase_core=gbase_core)
    per_core = []
    for c in range(NCORES):
        lo_pc = c * TOT_PC
        d = {
            "gidx": _wrap_idx(idx16[lo_pc:lo_pc + TOT_PC]),
            "grel": np.ascontiguousarray(
                rel[lo_pc:lo_pc + TOT_PC].reshape(-1, P).T),
            "didx": _wrap_idx(dsti16[lo_pc:lo_pc + TOT_PC]),
            "brel_own": batch_rel_block(c * NPC, gbase_core[c], P),
            "brel256": batch_rel_block(c * NPC, 0, 256),
        }
        per_core.append(d)
    return meta, per_core, dict(order=order, slot_of_edge=slot_of_edge,
                                grecip=grecip, gbase_core=gbase_core)


def pack_weights(inp):
    cols, wmap = [], {}

    def addw(name, m):
        m = np.asarray(m, np.float32)
        wmap[name] = (sum(c.shape[1] for c in cols), m.shape[1])
        cols.append(m.astype(np.float16))

    I64 = np.eye(64, dtype=np.float32)
    for l in range(L):
        addw(f"ce_W1_{l}", inp["ce_W1"][l]); addw(f"ce_W2_{l}", inp["ce_W2"][l])
        addw(f"ce_self_{l}", (1.0 + float(inp["ce_eps"][l])) * I64)
        addw(f"cl_W1_{l}", inp["cl_W1"][l]); addw(f"cl_W2_{l}", inp["cl_W2"][l])
        addw(f"cl_self_{l}", (1.0 + float(inp["cl_eps"][l])) * I64)
    for k in range(K):
        addw(f"nm_W1_{k}", inp["nm_W1"][k]); addw(f"nm_W2_{k}", inp["nm_W2"][k])
        addw(f"fm_W1_{k}", inp["fm_W1"][k]); addw(f"fm_W2_{k}", inp["fm_W2"][k])
        addw(f"em_W2_{k}", inp["em_W2"][k]); addw(f"clf_W_{k}", inp["clf_W"][k])
    em_W1 = np.asarray(inp["em_W1"], np.float32)
    for g in range(2):
        k0, k1 = 2 * g, 2 * g + 1
        addw(f"em_W1a_p{g}", np.concatenate([em_W1[k0, :64], em_W1[k1, :64]], 1))
        addw(f"em_W1b_p{g}", np.concatenate([em_W1[k0, 64:], em_W1[k1, 64:]], 1))
    wf16 = np.concatenate(cols, axis=1)

    # [128]-row blockdiag weights in their own pack
    qcols, qmap = [], {}
    for g in range(2):
        k0, k1 = 2 * g, 2 * g + 1
        for nm_, w2 in (("em", inp["em_W2"]), ("nm", inp["nm_W2"])):
            bd = np.zeros((P, 2), np.float32)
            bd[:64, 0] = np.asarray(w2[k0], np.float32).reshape(-1)
            bd[64:, 1] = np.asarray(w2[k1], np.float32).reshape(-1)
            qmap[f"{nm_}_W2bd_p{g}"] = (sum(c.shape[1] for c in qcols), 2)
            qcols.append(bd.astype(np.float16))
    qrows = [0, 1, 32, 33]
    for k in range(K):
        sel = np.zeros((P, 64), np.float32)
        sel[qrows[k], :] = 1.0
        qmap[f"sel_{k}"] = (sum(c.shape[1] for c in qcols), 64)
        qcols.append(sel.astype(np.float16))
    wq16 = np.concatenate(qcols, axis=1)

    bcols, bmap = [], {}

    def addb(name, v):
        col = np.zeros((P, 1), np.float32)
        v = np.asarray(v, np.float32).reshape(-1)
        col[: v.shape[0], 0] = v
        bmap[name] = len(bcols)
        bcols.append(col)

    for l in range(L):
        addb(f"ce_b1_{l}", inp["ce_b1"][l]); addb(f"ce_b2_{l}", inp["ce_b2"][l])
        addb(f"cl_b1_{l}", inp["cl_b1"][l]); addb(f"cl_b2_{l}", inp["cl_b2"][l])
    for k in range(K):
        addb(f"nm_b1_{k}", inp["nm_b1"][k])
        addb(f"fm_b1_{k}", inp["fm_b1"][k]); addb(f"fm_b2_{k}", inp["fm_b2"][k])
        addb(f"clf_b_{k}", inp["clf_b"][k])
    for g in range(2):
        addb(f"em_b1_p{g}", np.r_[np.asarray(inp["em_b1"][2 * g], np.float32),
                                  np.asarray(inp["em_b1"][2 * g + 1], np.float32)])
    for nm_, b2 in (("nm", inp["nm_b2"]), ("em", inp["em_b2"])):
        col = np.zeros(P, np.float32)
        for k in range(K):
            col[(k // 2) * 32 + k % 2] = float(np.asarray(b2[k]).reshape(-1)[0])
        addb(f"{nm_}_b2_q", col)
    bf32 = np.concatenate(bcols, axis=1)
    return wf16, wq16, bf32, wmap, qmap, bmap


# ------------------------------------------------------------- device program
def build_program(meta):
    R_LO, R_HI, RW = meta["R_LO"], meta["R_HI"], meta["RW"]
    SLOTW, TOT_PC = meta["SLOTW"], meta["TOT_PC"]
    gbase_core = meta["gbase_core"]
    wmap, bmap, qmap = meta["wmap"], meta["bmap"], meta["qmap"]
    wc, bc, qc = meta["wc"], meta["bc"], meta["qc"]
    EMROWS = RW * K           # wrapped ew cols per window: (r, k) order

    nc = bacc.Bacc("TRN2", target_bir_lowering=False, debug=False)
    D = nc.dram_tensor

    x_tab = D("x_tab", [NPAD, 2 * H], f16, kind="ExternalInput")
    xT_own = D("xT_own", [H, NPC], f16, kind="ExternalInput")
    gidx_d = D("gidx", [32, TOT_PC // 16], i16, kind="ExternalInput")
    grel_d = D("grel", [P, WPC * RW], f16, kind="ExternalInput")
    didx_d = D("didx", [32, TOT_PC // 16], i16, kind="ExternalInput")
    brelo_d = D("brel_own", [P, WPC], f16, kind="ExternalInput")
    brel2_d = D("brel256", [P, WPC], f16, kind="ExternalInput")
    iota_d = D("iota", [P, P], f16, kind="ExternalInput")
    iota2_d = D("iota256", [P, 256], f16, kind="ExternalInput")
    ident_d = D("ident", [P, P], f16, kind="ExternalInput")
    ones_d = D("ones64", [1, H], f16, kind="ExternalInput")
    wf_d = D("wf16", [H, wc], f16, kind="ExternalInput")
    wq_d = D("wq16", [P, qc], f16, kind="ExternalInput")
    bf_d = D("bf32", [P, bc], f32, kind="ExternalInput")
    grecip_d = D("grecip64", [H, B], f32, kind="ExternalInput")

    nm_out = D("nm_out", [K, NPC], f16, kind="ExternalOutput")
    fm_out = D("fm_out", [K, H, NPC], f16, kind="ExternalOutput")
    ew_out = D("ew_out", [P, WPC * EMROWS], f16, kind="ExternalOutput")
    hs_out = D("hs_out", [K, H, B], f32, kind="ExternalOutput")
    lg_out = D("lg_out", [K, 16, B], f32, kind="ExternalOutput")
    ho_out = D("ho_out", [H, B], f32, kind="ExternalOutput")

    with tile.TileContext(nc) as tc:
        with (
            tc.tile_pool(name="const", bufs=1) as cpool,
            tc.tile_pool(name="strip", bufs=1) as spool,
            tc.tile_pool(name="w1", bufs=1) as wp1,
            tc.tile_pool(name="w2", bufs=2) as wp2,
            tc.tile_pool(name="w3", bufs=2) as wp3,
            tc.tile_pool(name="psagg", bufs=4, space="PSUM") as psagg,
            tc.tile_pool(name="psmlp", bufs=2, space="PSUM") as psmlp,
            tc.tile_pool(name="pstp", bufs=2, space="PSUM") as pstp,
            tc.tile_pool(name="dram", bufs=1, space="DRAM") as dpool,
        ):
            # ---------------- constants
            grel = cpool.tile([P, WPC * RW], f16)
            iota = cpool.tile([P, P], f16)
            iota2 = cpool.tile([P, 256], f16)
            ident = cpool.tile([P, P], f16)
            ones = cpool.tile([1, H], f16)
            wf = cpool.tile([H, wc], f16)
            wq = cpool.tile([P, qc], f16)
            bf = cpool.tile([P, bc], f32)
            brelo = cpool.tile([P, WPC], f16)
            brel2 = cpool.tile([P, WPC], f16)
            grecip = cpool.tile([H, B], f32)
            xT = cpool.tile([H, NPC], f16)
            for t, d in ((grel, grel_d), (iota, iota_d), (iota2, iota2_d),
                         (ident, ident_d), (ones, ones_d), (wf, wf_d),
                         (wq, wq_d), (bf, bf_d), (brelo, brelo_d),
                         (brel2, brel2_d), (grecip, grecip_d), (xT, xT_own)):
                nc.sync.dma_start(t[:], d[:])

            def W(name):
                o, n = wmap[name]
                return wf[:, o:o + n]

            def Wq(name):
                o, n = qmap[name]
                return wq[:, o:o + n]

            def bias(name, p=64):
                return bf[:p, bmap[name]:bmap[name] + 1]

            def bias_q(name):
                # rows {0,1,32,33} as a [2,2,1] AP
                col = bmap[name]
                return bf[:].rearrange("(g q) c -> g q c", q=32)[0:2, 0:2,
                                                                col:col + 1]

            # ---------------- strips & staging
            sA = spool.tile([H, NPC], f16)
            sB = spool.tile([H, NPC], f16)
            aggT = spool.tile([H, NPC], f16)
            cldst = [spool.tile([H, NPC], f16, tag=f"clh{k}", name=f"clh{k}")
                     for k in range(K)]
            stagingP = spool.tile([P, WPC, 4 * H], f16)
            ew = spool.tile([P, WPC, RW, 2, 2], f16)
            nc.gpsimd.memset(stagingP[:, :, H:2 * H], 0.0)

            # ---------------- DRAM tables
            tabs = [dpool.tile([NPAD, 2 * H], f16, tag=f"tab{i}", name=f"tab{i}")
                    for i in range(2)]
            tabZ = dpool.tile([NPAD, 2 * H], f16)
            tabZo = dpool.tile([NPC, 2 * H], f16)
            tabP = [dpool.tile([NPAD, 4 * H], f16, tag=f"tabP{i}", name=f"tabP{i}")
                    for i in range(2)]
            payl = dpool.tile([NPC, 2 * H], f16)
            paylP = dpool.tile([NPC, 4 * H], f16)
            FLAT = K * H * B + H * P
            cc_in = dpool.tile([FLAT], f32)
            cc_out = dpool.tile([NCORES, FLAT], f32)

            def load_idx(w, src_d):
                t = wp2.tile([32, SLOTW // 16], i16, tag="idxw")
                nc.sync.dma_start(t[:], src_d[:, w * (SLOTW // 16):(w + 1) * (SLOTW // 16)])
                return t

            def build_S(w):
                s = wp2.tile([P, RW, P], f16, tag="S")
                nc.vector.tensor_tensor(
                    out=s[:],
                    in0=grel[:, w * RW:(w + 1) * RW, None].to_broadcast([P, RW, P]),
                    in1=iota[:, None, :].to_broadcast([P, RW, P]),
                    op=OP.is_equal)
                return s

            def mlp(dst, src_strip, w1, b1, w2, b2):
                for (o, n) in CHUNKS:
                    p1 = psmlp.tile([P, 512], f32, tag="mlp")
                    nc.tensor.matmul(p1[:H, :n], w1, src_strip[:, o:o + n],
                                     start=True, stop=True)
                    h1 = wp3.tile([H, 512], f16, tag="mlph")
                    nc.scalar.activation(h1[:, :n], p1[:H, :n], AF.Relu, bias=b1)
                    p2 = psmlp.tile([P, 512], f32, tag="mlp")
                    nc.tensor.matmul(p2[:H, :n], w2, h1[:, :n],
                                     start=True, stop=True)
                    nc.scalar.activation(dst[:, o:o + n], p2[:H, :n], AF.Relu,
                                         bias=b2)

            def transpose_to(dst_ap, src_ap, in_parts=H):
                pt = pstp.tile([P, P], f16, tag="tp")
                w_ = src_ap.shape[-1]
                nc.tensor.transpose(out=pt[:w_, :in_parts], in_=src_ap,
                                    identity=ident[:in_parts, :in_parts])
                nc.vector.tensor_copy(dst_ap, pt[:w_, :in_parts])

            def aggregate(table, packed, src_strip, selfW, dst_strip, use_ew=False, k=0):
                elem = 4 * H if packed else 2 * H
                for w in range(WPC):
                    gi = load_idx(w, gidx_d)
                    m = wp2.tile([P, RW, elem], f16,
                                 tag="msg")
                    nc.gpsimd.dma_gather(
                        m[:, :R_LO, :], table[:NLO], gi[:, :R_LO * 8],
                        R_LO * P, R_LO * P, elem, single_packet=False)
                    nc.gpsimd.dma_gather(
                        m[:, R_LO:, :], table[NLO:], gi[:, R_LO * 8:],
                        R_HI * P, R_HI * P, elem, single_packet=False)
                    if use_ew:
                        nc.vector.tensor_tensor(
                            out=m[:, :, k * H:(k + 1) * H],
                            in0=m[:, :, k * H:(k + 1) * H],
                            in1=ew[:, w, :, k // 2, k % 2, None]
                                .to_broadcast([P, RW, H]),
                            op=OP.mult)
                    s = build_S(w)
                    pa = psagg.tile([H, P], f32, tag="agg")
                    for r in range(RW):
                        nc.tensor.matmul(pa[:], m[:, r, k * H:k * H + H],
                                         s[:, r, :], start=(r == 0), stop=False)
                    nc.tensor.matmul(pa[:], selfW,
                                     src_strip[:, w * P:(w + 1) * P],
                                     start=False, stop=True)
                    nc.scalar.activation(dst_strip[:, w * P:(w + 1) * P], pa[:],
                                         AF.Copy)

            def ag8(in_t, out_t):
                nc.gpsimd.collective_compute(
                    "AllGather", OP.bypass,
                    replica_groups=[list(range(NCORES))],
                    ins=[in_t[:]], outs=[out_t[:]])

            # =================== ce GIN ===================
            strips = [xT, sA, sB, sA]
            for l in range(L):
                table = x_tab if l == 0 else tabs[(l + 1) % 2]
                aggregate(table, False, strips[l], W(f"ce_self_{l}"), aggT)
                mlp(strips[l + 1], aggT, W(f"ce_W1_{l}"), bias(f"ce_b1_{l}"),
                    W(f"ce_W2_{l}"), bias(f"ce_b2_{l}"))
                for w in range(WPC):
                    transpose_to(stagingP[:, w, :H],
                                 strips[l + 1][:, w * P:(w + 1) * P])
                nc.sync.dma_start(
                    payl[:].rearrange("(w p) e -> p w e", p=P),
                    stagingP[:, :, :2 * H])
                ag8(payl, tabZ if l == L - 1 else tabs[l % 2])
                if l == L - 1:
                    nc.sync.dma_start(tabZo[:], payl[:])
            ZT = strips[L]   # = sA

            # =================== h_orig partial ===================
            pho = psagg.tile([H, P], f32, tag="agg")
            for w in range(WPC):
                sb = wp2.tile([P, P], f16, tag="sbw")
                nc.vector.tensor_tensor(
                    out=sb[:], in0=brelo[:, w:w + 1].to_broadcast([P, P]),
                    in1=iota[:], op=OP.is_equal)
                nc.tensor.matmul(pho[:], stagingP[:, w, :H], sb[:],
                                 start=(w == 0), stop=(w == WPC - 1))
            ho_part = spool.tile([H, P], f32)
            nc.vector.tensor_copy(ho_part[:], pho[:])

            # =================== masks + masked_x ===================
            for (o, n) in CHUNKS:
                pnm = pstp.tile([P, 512], f32, tag="tp")
                for g in range(2):
                    k0, k1 = 2 * g, 2 * g + 1
                    p1 = psmlp.tile([P, 512], f32, tag="mlp")
                    nc.tensor.matmul(p1[:H, :n], W(f"nm_W1_{k0}"),
                                     ZT[:, o:o + n], start=True, stop=True)
                    nc.tensor.matmul(p1[H:, :n], W(f"nm_W1_{k1}"),
                                     ZT[:, o:o + n], start=True, stop=True)
                    h1 = wp3.tile([P, 512], f16, tag="mlph")
                    nc.scalar.activation(h1[:H, :n], p1[:H, :n], AF.Relu,
                                         bias=bias(f"nm_b1_{k0}"))
                    nc.scalar.activation(h1[H:, :n], p1[H:, :n], AF.Relu,
                                         bias=bias(f"nm_b1_{k1}"))
                    nc.tensor.matmul(pnm[32 * g:32 * g + 2, :n],
                                     Wq(f"nm_W2bd_p{g}"), h1[:, :n],
                                     start=True, stop=True)
                nmc = wp2.tile([34, 512], f16, tag="nmc")
                nc.scalar.activation(nmc[:, :n], pnm[:34, :n], AF.Sigmoid,
                                     bias=bias("nm_b2_q", 34))
                nc.sync.dma_start(nm_out[0:2, o:o + n], nmc[0:2, :n])
                nc.sync.dma_start(nm_out[2:4, o:o + n], nmc[32:34, :n])
                for k in range(K):
                    p2 = psmlp.tile([P, 512], f32, tag="mlp")
                    nc.tensor.matmul(p2[:H, :n], W(f"fm_W1_{k}"), ZT[:, o:o + n],
                                     start=True, stop=True)
                    h2 = wp3.tile([P, 512], f16, tag="mlph")
                    nc.scalar.activation(h2[:H, :n], p2[:H, :n], AF.Relu,
                                         bias=bias(f"fm_b1_{k}"))
                    p3 = psmlp.tile([P, 512], f32, tag="mlp")
                    nc.tensor.matmul(p3[:H, :n], W(f"fm_W2_{k}"), h2[:H, :n],
                                     start=True, stop=True)
                    fk = wp3.tile([P, 512], f16, tag="mlph")
                    nc.scalar.activation(fk[:H, :n], p3[:H, :n], AF.Sigmoid,
                                         bias=bias(f"fm_b2_{k}"))
                    nc.sync.dma_start(fm_out[k, :, o:o + n], fk[:H, :n])
                    pb = psmlp.tile([P, 512], f32, tag="mlp")
                    nc.tensor.matmul(pb[:H, :n], Wq(f"sel_{k}")[:34, :],
                                     nmc[:, :n], start=True, stop=True)
                    nmb = wp3.tile([P, 512], f16, tag="mlph")
                    nc.vector.tensor_copy(nmb[:H, :n], pb[:H, :n])
                    nc.vector.tensor_tensor(out=cldst[k][:, o:o + n],
                                            in0=xT[:, o:o + n],
                                            in1=fk[:H, :n], op=OP.mult)
                    nc.vector.tensor_tensor(out=cldst[k][:, o:o + n],
                                            in0=cldst[k][:, o:o + n],
                                            in1=nmb[:H, :n], op=OP.mult)
                for j in range(n // P):
                    w = (o + j * P) // P
                    for k in range(K):
                        transpose_to(stagingP[:, w, k * H:(k + 1) * H],
                                     cldst[k][:, w * P:(w + 1) * P])
            nc.sync.dma_start(
                paylP[:].rearrange("(w p) e -> p w e", p=P), stagingP[:])
            ag8(paylP, tabP[0])

            # =================== edge masks ===================
            # chunk plan over one window's slots: (src tile key, local off,
            # global off, len)
            emplan = []
            for ch in range(0, R_LO * P, 512):
                emplan.append(("L", ch, ch, min(512, R_LO * P - ch)))
            for ch in range(0, R_HI * P, 512):
                emplan.append(("Hh", ch, R_LO * P + ch, min(512, R_HI * P - ch)))
            for w in range(WPC):
                gi = load_idx(w, gidx_d)
                di = load_idx(w, didx_d)
                zsL = wp1.tile([P, 1, R_LO * P], f16, tag="zsL")
                zsH = wp1.tile([P, 1, R_HI * P], f16, tag="zsH")
                zd = wp1.tile([P, 1, SLOTW], f16, tag="zd")
                nc.gpsimd.dma_gather(zsL[:], tabZ[:NLO], gi[:, :R_LO * 8],
                                     R_LO * P, R_LO * P, 2 * H, transpose=True,
                                     single_packet=False)
                nc.gpsimd.dma_gather(zsH[:], tabZ[NLO:], gi[:, R_LO * 8:],
                                     R_HI * P, R_HI * P, 2 * H, transpose=True,
                                     single_packet=False)
                nc.gpsimd.dma_gather(zd[:], tabZo[:], di[:],
                                     SLOTW, SLOTW, 2 * H, transpose=True,
                                     single_packet=False)
                eo = wp1.tile([34, SLOTW], f16, tag="ewsig")
                for (skey, lo, go, n) in emplan:
                    zs = zsL if skey == "L" else zsH
                    po = psagg.tile([P, 512], f32, tag="agg")
                    for g in range(2):
                        ph2 = psmlp.tile([P, 512], f32, tag="mlp")
                        nc.tensor.matmul(ph2[:, :n], W(f"em_W1a_p{g}"),
                                         zs[:H, 0, lo:lo + n], start=True,
                                         stop=False)
                        nc.tensor.matmul(ph2[:, :n], W(f"em_W1b_p{g}"),
                                         zd[:H, 0, go:go + n], start=False,
                                         stop=True)
                        hh = wp3.tile([P, 512], f16, tag="mlph")
                        if g == 0:
                            nc.scalar.activation(hh[:, :n], ph2[:, :n], AF.Relu,
                                                 bias=bias("em_b1_p0", P))
                        else:
                            nc.vector.tensor_scalar(
                                out=hh[:, :n], in0=ph2[:, :n],
                                scalar1=bias("em_b1_p1", P), scalar2=0.0,
                                op0=OP.add, op1=OP.max)
                        nc.tensor.matmul(po[32 * g:32 * g + 2, :n],
                                         Wq(f"em_W2bd_p{g}"), hh[:, :n],
                                         start=True, stop=True)
                    nc.scalar.activation(eo[:, go:go + n], po[:34, :n],
                                         AF.Sigmoid, bias=bias("em_b2_q", 34))
                # wrap [34, SLOTW] -> ew[:, w, r, g, q] via RW transposes
                pt = pstp.tile([P, RW * 34], f16, tag="tp")
                for r in range(RW):
                    nc.tensor.transpose(out=pt[:, 34 * r:34 * r + 34],
                                        in_=eo[:, r * P:(r + 1) * P],
                                        identity=ident[:34, :34])
                ptv = pt[:].rearrange("p (r e) -> p r e", r=RW)
                nc.vector.tensor_copy(ew[:, w, :, 0, :], ptv[:, :, 0:2])
                nc.vector.tensor_copy(ew[:, w, :, 1, :], ptv[:, :, 32:34])
            nc.sync.dma_start(ew_out[:], ew[:].rearrange("p w r g q -> p (w r g q)"))

            # =================== cl GIN ===================
            for l in range(L):
                table = tabP[l % 2]
                selfW = W(f"cl_self_{l}")
                for w in range(WPC):
                    gi = load_idx(w, gidx_d)
                    m = wp2.tile([P, RW, 4 * H], f16, tag="msg")
                    nc.gpsimd.dma_gather(
                        m[:, :R_LO, :], table[:NLO], gi[:, :R_LO * 8],
                        R_LO * P, R_LO * P, 4 * H, single_packet=False)
                    nc.gpsimd.dma_gather(
                        m[:, R_LO:, :], table[NLO:], gi[:, R_LO * 8:],
                        R_HI * P, R_HI * P, 4 * H, single_packet=False)
                    s = build_S(w)
                    for k in range(K):
                        nc.vector.tensor_tensor(
                            out=m[:, :, k * H:(k + 1) * H],
                            in0=m[:, :, k * H:(k + 1) * H],
                            in1=ew[:, w, :, k // 2, k % 2, None]
                                .to_broadcast([P, RW, H]),
                            op=OP.mult)
                        pa = psagg.tile([H, P], f32, tag="agg",
                                        name=f"clpa{k}")
                        for r in range(RW):
                            nc.tensor.matmul(pa[:], m[:, r, k * H:k * H + H],
                                             s[:, r, :], start=(r == 0),
                                             stop=False)
                        nc.tensor.matmul(pa[:], selfW,
                                         cldst[k][:, w * P:(w + 1) * P],
                                         start=False, stop=True)
                        nc.scalar.activation(
                            cldst[k][:, w * P:(w + 1) * P], pa[:], AF.Copy)
                for k in range(K):
                    mlp(cldst[k], cldst[k], W(f"cl_W1_{l}"), bias(f"cl_b1_{l}"),
                        W(f"cl_W2_{l}"), bias(f"cl_b2_{l}"))
                    for w in range(WPC):
                        transpose_to(stagingP[:, w, k * H:(k + 1) * H],
                                     cldst[k][:, w * P:(w + 1) * P])
                if l < L - 1:
                    nc.sync.dma_start(
                        paylP[:].rearrange("(w p) e -> p w e", p=P), stagingP[:])
                    ag8(paylP, tabP[(l + 1) % 2])

            # =================== pooling, h_stable, logits, h_orig ========
            pk = [psagg.tile([H, 256], f32, tag="agg", name=f"pk{k}") for k in range(K)]
            for w in range(WPC):
                sb2 = wp2.tile([P, 256], f16, tag="sb2")
                nc.vector.tensor_tensor(
                    out=sb2[:], in0=brel2[:, w:w + 1].to_broadcast([P, 256]),
                    in1=iota2[:], op=OP.is_equal)
                for k in range(K):
                    nc.tensor.matmul(pk[k][:], stagingP[:, w, k * H:(k + 1) * H],
                                     sb2[:], start=(w == 0), stop=(w == WPC - 1))
            for k in range(K):
                pp = wp2.tile([H, 256], f32, tag="poolp")
                nc.vector.tensor_copy(pp[:], pk[k][:])
                nc.sync.dma_start(
                    cc_in[k * H * B:(k + 1) * H * B]
                    .rearrange("(p e) -> p e", p=H), pp[:])
            nc.sync.dma_start(
                cc_in[K * H * B:].rearrange("(p e) -> p e", p=H), ho_part[:])
            ag8(cc_in, cc_out)

            # combine pools
            for k in range(K):
                acc = wp2.tile([H, 256], f32, tag="acc")
                tmp = wp1.tile([H, NCORES, 256], f32, tag="ptmp")
                nc.sync.dma_start(
                    tmp[:],
                    cc_out[:, k * H * B:(k + 1) * H * B]
                    .rearrange("c (p e) -> p c e", p=H))
                nc.vector.tensor_tensor(out=acc[:], in0=tmp[:, 0, :],
                                        in1=tmp[:, 1, :], op=OP.add)
                for j in range(2, NCORES):
                    nc.vector.tensor_tensor(out=acc[:], in0=acc[:],
                                            in1=tmp[:, j, :], op=OP.add)
                nc.vector.tensor_tensor(out=acc[:], in0=acc[:], in1=grecip[:],
                                        op=OP.mult)
                nc.sync.dma_start(hs_out[k], acc[:])
                h16 = wp1.tile([H, 256], f16, tag="h16")
                nc.vector.tensor_copy(h16[:], acc[:])
                pl = pstp.tile([16, 256], f32, tag="tp")
                nc.tensor.matmul(pl[:C, :], W(f"clf_W_{k}"), h16[:],
                                 start=True, stop=True)
                lgt = wp1.tile([16, 256], f32, tag="lgt")
                nc.vector.tensor_scalar(out=lgt[:C, :], in0=pl[:C, :],
                                        scalar1=bias(f"clf_b_{k}", C),
                                        scalar2=None, op0=OP.add)
                nc.sync.dma_start(lg_out[k, :C, :], lgt[:C, :])

            # combine h_orig
            hof = wp2.tile([H, 256], f32, tag="acc")
            nc.gpsimd.memset(hof[:], 0.0)
            hot = wp1.tile([H, NCORES, P], f32, tag="hot")
            nc.sync.dma_start(
                hot[:], cc_out[:, K * H * B:].rearrange("c (p e) -> p c e", p=H))
            for j in range(NCORES):
                g0 = gbase_core[j]
                nwid = min(P, B - g0)
                nc.vector.tensor_tensor(
                    out=hof[:, g0:g0 + nwid], in0=hof[:, g0:g0 + nwid],
                    in1=hot[:, j, :nwid], op=OP.add)
            nc.vector.tensor_tensor(out=hof[:], in0=hof[:], in1=grecip[:],
                                    op=OP.mult)
            nc.sync.dma_start(ho_out[:], hof[:])

    nc.compile()
    return nc


# ---------------------------------------------------------------- entry point
def make_in_maps(inputs, meta, per_core, post, wf16, wq16, bf32):
    x = np.asarray(inputs["x"], np.float32)
    xpad = np.zeros((NPAD, 2 * H), np.float16)
    xpad[:N, :H] = x.astype(np.float16)
    iota = np.tile(np.arange(P, dtype=np.float16), (P, 1))
    iota2 = np.tile(np.arange(256, dtype=np.float16), (P, 1))
    ident = np.eye(P, dtype=np.float16)
    ones64 = np.ones((1, H), np.float16)
    grecip64 = np.tile(post["grecip"].reshape(1, B), (H, 1)).astype(np.float32)

    in_maps = []
    for c in range(NCORES):
        d = per_core[c]
        xsl = np.zeros((H, NPC), np.float16)
        hi = min((c + 1) * NPC, N)
        xsl[:, : hi - c * NPC] = x[c * NPC:hi].T.astype(np.float16)
        in_maps.append({
            "x_tab": xpad, "xT_own": xsl,
            "gidx": d["gidx"], "grel": d["grel"], "didx": d["didx"],
            "brel_own": d["brel_own"], "brel256": d["brel256"],
            "iota": iota, "iota256": iota2, "ident": ident, "ones64": ones64,
            "wf16": wf16, "wq16": wq16, "bf32": bf32, "grecip64": grecip64,
        })
    return in_maps


def kernel(**inputs):
    x = np.asarray(inputs["x"], np.float32)
    meta, per_core, post = prep(x, inputs["edge_index"], inputs["batch"])
    wf16, wq16, bf32, wmap, qmap, bmap = pack_weights(inputs)
    meta["wmap"], meta["bmap"], meta["qmap"] = wmap, bmap, qmap
    meta["wc"], meta["bc"], meta["qc"] = (wf16.shape[1], bf32.shape[1],
                                          wq16.shape[1])
    nc = build_program(meta)
    in_maps = make_in_maps(inputs, meta, per_core, post, wf16, wq16, bf32)
    res = run_bass_kernel_spmd(nc, in_maps, core_ids=list(range(NCORES)))
    return assemble(res.results, meta, post)


def assemble(results, meta, post):
    RW = meta["RW"]
    # node-split outputs
    nm = np.concatenate([results[c]["nm_out"] for c in range(NCORES)], axis=1)
    fm = np.concatenate([results[c]["fm_out"] for c in range(NCORES)], axis=2)
    node_masks = nm.astype(np.float32).T[:N, :, None]            # [N, K, 1]
    feat_masks = fm.astype(np.float32).transpose(2, 0, 1)[:N]    # [N, K, F]

    # edge masks: ew_out [128, WPC*EMROWS]; value for (p, w, k, r) is slot
    # w*SLOTW + r*128 + p of that core.
    EMR = K * RW
    ews = np.stack([results[c]["ew_out"] for c in range(NCORES)])  # [8,128,WPC*EMR]
    ews = ews.reshape(NCORES, P, WPC, RW, K)
    soe = post["slot_of_edge"]
    c_e = soe // meta["TOT_PC"]
    rem = soe % meta["TOT_PC"]
    w_e = rem // meta["SLOTW"]
    r_e = (rem % meta["SLOTW"]) // P
    p_e = rem % P
    edge_masks = np.empty((E, K, 1), np.float32)
    for k in range(K):
        edge_masks[:, k, 0] = ews[c_e, p_e, w_e, r_e, k]

    hs = results[0]["hs_out"]          # [K, H, B]
    lg = results[0]["lg_out"][:, :C]   # [K, C, B]
    ho = results[0]["ho_out"]          # [H, B]
    expert_logits = lg.transpose(2, 0, 1).astype(np.float32)
    h_stable = hs.transpose(2, 0, 1).astype(np.float32)
    h_orig = ho.T.astype(np.float32)
    return (expert_logits, h_stable, h_orig, node_masks, edge_masks, feat_masks)


# revision 28
# speedup vs baseline: 2.1252x; 1.1447x over previous
"""Trainium2 Bass kernel for nn_Experts_66614942761445 (GNN message passing,
K-expert masked GIN). Self-contained: host-side numpy preprocessing + an SPMD
Bass/Tile program on 8 NeuronCores + host-side output assembly.

All phases are node-split: core c owns nodes [6272c, 6272c+6272) (padded to
50176). Segment-sum aggregation runs windowed: for each window of 128
destination slots, dma_gather the source rows (int16 idx, LO/HI split at
32768), build S[e,seg] = (rel[e]==seg) on DVE in fp16, and accumulate
aggT[:,win] += msg.T @ S on PE into f32 PSUM. Node tables are f16 node-major
256B rows (cl tables pack all 4 experts per row -> 512B, one descriptor
serves 4 experts). Tables are rebuilt each layer with an 8-core AllGather.
The edge-mask MLP reuses the same gather streams (transpose-mode gathers give
feature-major Z) and keeps edge weights on-chip, wrapped per window with one
PE transpose. Pooling is an S-matmul against batch membership.
"""
import math
import numpy as np

import concourse.bass as bass
import concourse.bacc as bacc
import concourse.mybir as mybir
import concourse.tile as tile
from concourse.bass_utils import run_bass_kernel_spmd

N, E, H, K, L, B, C = 50000, 800000, 64, 4, 3, 256, 10
P = 128
NCORES = 8
NPC = 6272
WPC = NPC // P            # 49
NPAD = 8 * NPC            # 50176
NW = 8 * WPC              # 392
NLO = 32768
NHI = NPAD - NLO          # 17408

f32 = mybir.dt.float32
f16 = mybir.dt.float16
i16 = mybir.dt.int16
AF = mybir.ActivationFunctionType
OP = mybir.AluOpType

CHUNKS = [(i * 512, 512) for i in range(12)] + [(12 * 512, 128)]


# ----------------------------------------------------------------- host prep
def _wrap_idx(idx):
    n = idx.shape[0]
    t = idx.reshape(n // 16, 16).T
    return np.tile(t, (2, 1)).copy()


def _group_positions(sorted_keys):
    n = sorted_keys.shape[0]
    pos = np.arange(n, dtype=np.int64)
    starts = np.r_[0, np.flatnonzero(np.diff(sorted_keys)) + 1]
    return pos - np.repeat(starts, np.diff(np.r_[starts, n]))


def prep(x, edge_index, batch):
    src = np.asarray(edge_index[0], dtype=np.int64)
    dst = np.asarray(edge_index[1], dtype=np.int64)
    batch = np.asarray(batch, dtype=np.int64)
    x = np.asarray(x, dtype=np.float32)

    w = dst // P
    is_hi = (src >= NLO).astype(np.int64)
    key = w * 2 + is_hi
    order = np.argsort(key, kind="stable")
    jpos = _group_positions(key[order])
    s_src, s_dst, s_w, s_hi = src[order], dst[order], w[order], is_hi[order]

    cnt = np.bincount(key, minlength=2 * NW)
    R_LO = int(math.ceil(cnt[0::2].max() / P))
    R_HI = int(math.ceil(cnt[1::2].max() / P))
    RW = R_LO + R_HI
    SLOTW = RW * P
    TOT = NW * SLOTW
    TOT_PC = WPC * SLOTW

    slots = s_w * SLOTW + np.where(s_hi == 1, R_LO * P + jpos, jpos)
    spread = np.arange(TOT, dtype=np.int64) * 40503
    r_chunk = (np.arange(TOT, dtype=np.int64) // P) % RW
    idx16 = np.where(r_chunk < R_LO, spread % NLO, spread % NHI).astype(np.int16)
    idx16[slots] = np.where(s_hi == 1, s_src - NLO, s_src).astype(np.int16)
    rel = np.full(TOT, -1.0, np.float16)
    rel[slots] = (s_dst - s_w * P).astype(np.float16)
    dsti16 = (spread % NPC).astype(np.int16)
    dsti16[slots] = (s_dst - (s_w // WPC) * NPC).astype(np.int16)
    slot_of_edge = np.empty(E, np.int64)
    slot_of_edge[order] = slots

    gcount = np.bincount(batch, minlength=B).astype(np.float32)
    grecip = (1.0 / np.maximum(gcount, 1.0)).astype(np.float32)
    gbase_core = [int(batch[min(c * NPC, N - 1)]) for c in range(NCORES)]

    def batch_rel_block(lo, gbase, width):
        nb = np.full(NPC, -1.0, np.float32)
        hi = min(lo + NPC, N)
        if hi > lo:
            val = batch[lo:hi] - gbase
            val = np.where((val >= 0) & (val < width), val, -1)
            nb[: hi - lo] = val
        return np.ascontiguousarray(nb.reshape(WPC, P).T.astype(np.float16))

    meta = dict(R_LO=R_LO, R_HI=R_HI, RW=RW, SLOTW=SLOTW, TOT_PC=TOT_PC,
                gbase_core=gbase_core)
    per_core = []
    for c in range(NCORES):
        lo_pc = c * TOT_PC
        d = {
            "gidx": _wrap_idx(idx16[lo_pc:lo_pc + TOT_PC]),
            "grel": np.ascontiguousarray(
                rel[lo_pc:lo_pc + TOT_PC].reshape(-1, P).T),
            "didx": _wrap_idx(dsti16[lo_pc:lo_pc + TOT_PC]),
            "brel_own": batch_rel_block(c * NPC, gbase_core[c], P),
            "brel256": batch_rel_block(c * NPC, 0, 256),
        }
        per_core.append(d)
    return meta, per_core, dict(order=order, slot_of_edge=slot_of_edge,
                                grecip=grecip, gbase_core=gbase_core)


def pack_weights(inp):
    cols, wmap = [], {}

    def addw(name, m):
        m = np.asarray(m, np.float32)
        wmap[name] = (sum(c.shape[1] for c in cols), m.shape[1])
        cols.append(m.astype(np.float16))

    I64 = np.eye(64, dtype=np.float32)
    for l in range(L):
        addw(f"ce_W1_{l}", inp["ce_W1"][l]); addw(f"ce_W2_{l}", inp["ce_W2"][l])
        addw(f"ce_self_{l}", (1.0 + float(inp["ce_eps"][l])) * I64)
        addw(f"cl_W1_{l}", inp["cl_W1"][l]); addw(f"cl_W2_{l}", inp["cl_W2"][l])
        addw(f"cl_self_{l}", (1.0 + float(inp["cl_eps"][l])) * I64)
    for k in range(K):
        addw(f"nm_W1_{k}", inp["nm_W1"][k]); addw(f"nm_W2_{k}", inp["nm_W2"][k])
        addw(f"fm_W1_{k}", inp["fm_W1"][k]); addw(f"fm_W2_{k}", inp["fm_W2"][k])
        addw(f"em_W2_{k}", inp["em_W2"][k]); addw(f"clf_W_{k}", inp["clf_W"][k])
    em_W1 = np.asarray(inp["em_W1"], np.float32)
    for g in range(2):
        k0, k1 = 2 * g, 2 * g + 1
        addw(f"em_W1a_p{g}", np.concatenate([em_W1[k0, :64], em_W1[k1, :64]], 1))
        addw(f"em_W1b_p{g}", np.concatenate([em_W1[k0, 64:], em_W1[k1, 64:]], 1))
    wf16 = np.concatenate(cols, axis=1)

    # [128]-row blockdiag weights in their own pack
    qcols, qmap = [], {}
    for g in range(2):
        k0, k1 = 2 * g, 2 * g + 1
        for nm_, w2 in (("em", inp["em_W2"]), ("nm", inp["nm_W2"])):
            bd = np.zeros((P, 2), np.float32)
            bd[:64, 0] = np.asarray(w2[k0], np.float32).reshape(-1)
            bd[64:, 1] = np.asarray(w2[k1], np.float32).reshape(-1)
            qmap[f"{nm_}_W2bd_p{g}"] = (sum(c.shape[1] for c in qcols), 2)
            qcols.append(bd.astype(np.float16))
    qrows = [0, 1, 32, 33]
    for k in range(K):
        sel = np.zeros((P, 64), np.float32)
        sel[qrows[k], :] = 1.0
        qmap[f"sel_{k}"] = (sum(c.shape[1] for c in qcols), 64)
        qcols.append(sel.astype(np.float16))
    wq16 = np.concatenate(qcols, axis=1)

    bcols, bmap = [], {}

    def addb(name, v):
        col = np.zeros((P, 1), np.float32)
        v = np.asarray(v, np.float32).reshape(-1)
        col[: v.shape[0], 0] = v
        bmap[name] = len(bcols)
        bcols.append(col)

    for l in range(L):
        addb(f"ce_b1_{l}", inp["ce_b1"][l]); addb(f"ce_b2_{l}", inp["ce_b2"][l])
        addb(f"cl_b1_{l}", inp["cl_b1"][l]); addb(f"cl_b2_{l}", inp["cl_b2"][l])
    for k in range(K):
        addb(f"nm_b1_{k}", inp["nm_b1"][k])
        addb(f"fm_b1_{k}", inp["fm_b1"][k]); addb(f"fm_b2_{k}", inp["fm_b2"][k])
        addb(f"clf_b_{k}", inp["clf_b"][k])
    for g in range(2):
        addb(f"em_b1_p{g}", np.r_[np.asarray(inp["em_b1"][2 * g], np.float32),
                                  np.asarray(inp["em_b1"][2 * g + 1], np.float32)])
    for nm_, b2 in (("nm", inp["nm_b2"]), ("em", inp["em_b2"])):
        col = np.zeros(P, np.float32)
        for k in range(K):
            col[(k // 2) * 32 + k % 2] = float(np.asarray(b2[k]).reshape(-1)[0])
        addb(f"{nm_}_b2_q", col)
    bf32 = np.concatenate(bcols, axis=1)
    return wf16, wq16, bf32, wmap, qmap, bmap


# ------------------------------------------------------------- device program
def build_program(meta):
    R_LO, R_HI, RW = meta["R_LO"], meta["R_HI"], meta["RW"]
    SLOTW, TOT_PC = meta["SLOTW"], meta["TOT_PC"]
    gbase_core = meta["gbase_core"]
    wmap, bmap, qmap = meta["wmap"], meta["bmap"], meta["qmap"]
    wc, bc, qc = meta["wc"], meta["bc"], meta["qc"]
    EMROWS = RW * K           # wrapped ew cols per window: (r, k) order

    nc = bacc.Bacc("TRN2", target_bir_lowering=False, debug=False)
    D = nc.dram_tensor

    x_tab = D("x_tab", [NPAD, 2 * H], f16, kind="ExternalInput")
    xT_own = D("xT_own", [H, NPC], f16, kind="ExternalInput")
    gidx_d = D("gidx", [32, TOT_PC // 16], i16, kind="ExternalInput")
    grel_d = D("grel", [P, WPC * RW], f16, kind="ExternalInput")
    didx_d = D("didx", [32, TOT_PC // 16], i16, kind="ExternalInput")
    brelo_d = D("brel_own", [P, WPC], f16, kind="ExternalInput")
    brel2_d = D("brel256", [P, WPC], f16, kind="ExternalInput")
    iota_d = D("iota", [P, P], f16, kind="ExternalInput")
    iota2_d = D("iota256", [P, 256], f16, kind="ExternalInput")
    ident_d = D("ident", [P, P], f16, kind="ExternalInput")
    ones_d = D("ones64", [1, H], f16, kind="ExternalInput")
    wf_d = D("wf16", [H, wc], f16, kind="ExternalInput")
    wq_d = D("wq16", [P, qc], f16, kind="ExternalInput")
    bf_d = D("bf32", [P, bc], f32, kind="ExternalInput")
    grecip_d = D("grecip64", [H, B], f32, kind="ExternalInput")

    nm_out = D("nm_out", [K, NPC], f16, kind="ExternalOutput")
    fm_out = D("fm_out", [K, H, NPC], f16, kind="ExternalOutput")
    ew_out = D("ew_out", [P, WPC * EMROWS], f16, kind="ExternalOutput")
    hs_out = D("hs_out", [K, H, B], f32, kind="ExternalOutput")
    lg_out = D("lg_out", [K, 16, B], f32, kind="ExternalOutput")
    ho_out = D("ho_out", [H, B], f32, kind="ExternalOutput")

    with tile.TileContext(nc) as tc:
        with (
            tc.tile_pool(name="const", bufs=1) as cpool,
            tc.tile_pool(name="strip", bufs=1) as spool,
            tc.tile_pool(name="w1", bufs=1) as wp1,
            tc.tile_pool(name="w2", bufs=2) as wp2,
            tc.tile_pool(name="w3", bufs=2) as wp3,
            tc.tile_pool(name="psagg", bufs=4, space="PSUM") as psagg,
            tc.tile_pool(name="psmlp", bufs=2, space="PSUM") as psmlp,
            tc.tile_pool(name="pstp", bufs=2, space="PSUM") as pstp,
            tc.tile_pool(name="dram", bufs=1, space="DRAM") as dpool,
        ):
            # ---------------- constants
            grel = cpool.tile([P, WPC * RW], f16)
            iota = cpool.tile([P, P], f16)
            iota2 = cpool.tile([P, 256], f16)
            ident = cpool.tile([P, P], f16)
            ones = cpool.tile([1, H], f16)
            wf = cpool.tile([H, wc], f16)
            wq = cpool.tile([P, qc], f16)
            bf = cpool.tile([P, bc], f32)
            brelo = cpool.tile([P, WPC], f16)
            brel2 = cpool.tile([P, WPC], f16)
            grecip = cpool.tile([H, B], f32)
            xT = cpool.tile([H, NPC], f16)
            for t, d in ((grel, grel_d), (iota, iota_d), (iota2, iota2_d),
                         (ident, ident_d), (ones, ones_d), (wf, wf_d),
                         (wq, wq_d), (bf, bf_d), (brelo, brelo_d),
                         (brel2, brel2_d), (grecip, grecip_d), (xT, xT_own)):
                nc.sync.dma_start(t[:], d[:])

            def W(name):
                o, n = wmap[name]
                return wf[:, o:o + n]

            def Wq(name):
                o, n = qmap[name]
                return wq[:, o:o + n]

            def bias(name, p=64):
                return bf[:p, bmap[name]:bmap[name] + 1]

            def bias_q(name):
                # rows {0,1,32,33} as a [2,2,1] AP
                col = bmap[name]
                return bf[:].rearrange("(g q) c -> g q c", q=32)[0:2, 0:2,
                                                                col:col + 1]

            # ---------------- strips & staging
            sA = spool.tile([H, NPC], f16)
            sB = spool.tile([H, NPC], f16)
            aggT = spool.tile([H, NPC], f16)
            cldst = [spool.tile([H, NPC], f16, tag=f"clh{k}", name=f"clh{k}")
                     for k in range(K)]
            stagingP = spool.tile([P, WPC, 4 * H], f16)
            ew = spool.tile([P, WPC, RW, 2, 2], f16)
            nc.gpsimd.memset(stagingP[:, :, H:2 * H], 0.0)

            # ---------------- DRAM tables
            tabs = [dpool.tile([NPAD, 2 * H], f16, tag=f"tab{i}", name=f"tab{i}")
                    for i in range(2)]
            tabZ = dpool.tile([NPAD, 2 * H], f16)
            tabZo = dpool.tile([NPC, 2 * H], f16)
            tabP = [dpool.tile([NPAD, 4 * H], f16, tag=f"tabP{i}", name=f"tabP{i}")
                    for i in range(2)]
            payl = dpool.tile([NPC, 2 * H], f16)
            paylP = dpool.tile([NPC, 4 * H], f16)
            FLAT = K * H * B + H * P
            cc_in = dpool.tile([FLAT], f32)
            cc_out = dpool.tile([NCORES, FLAT], f32)

            def load_idx(w, src_d):
                t = wp2.tile([32, SLOTW // 16], i16, tag="idxw")
                nc.sync.dma_start(t[:], src_d[:, w * (SLOTW // 16):(w + 1) * (SLOTW // 16)])
                return t

            def build_S(w):
                s = wp2.tile([P, RW, P], f16, tag="S")
                nc.vector.tensor_tensor(
                    out=s[:],
                    in0=grel[:, w * RW:(w + 1) * RW, None].to_broadcast([P, RW, P]),
                    in1=iota[:, None, :].to_broadcast([P, RW, P]),
                    op=OP.is_equal)
                return s

            def mlp(dst, src_strip, w1, b1, w2, b2):
                for (o, n) in CHUNKS:
                    p1 = psmlp.tile([P, 512], f32, tag="mlp")
                    nc.tensor.matmul(p1[:H, :n], w1, src_strip[:, o:o + n],
                                     start=True, stop=True)
                    h1 = wp3.tile([H, 512], f16, tag="mlph")
                    nc.scalar.activation(h1[:, :n], p1[:H, :n], AF.Relu, bias=b1)
                    p2 = psmlp.tile([P, 512], f32, tag="mlp")
                    nc.tensor.matmul(p2[:H, :n], w2, h1[:, :n],
                                     start=True, stop=True)
                    nc.scalar.activation(dst[:, o:o + n], p2[:H, :n], AF.Relu,
                                         bias=b2)

            def transpose_to(dst_ap, src_ap, in_parts=H):
                pt = pstp.tile([P, P], f16, tag="tp")
                w_ = src_ap.shape[-1]
                nc.tensor.transpose(out=pt[:w_, :in_parts], in_=src_ap,
                                    identity=ident[:in_parts, :in_parts])
                nc.vector.tensor_copy(dst_ap, pt[:w_, :in_parts])

            def aggregate(table, packed, src_strip, selfW, dst_strip, use_ew=False, k=0):
                elem = 4 * H if packed else 2 * H
                for w in range(WPC):
                    gi = load_idx(w, gidx_d)
                    m = wp2.tile([P, RW, elem], f16,
                                 tag="msg")
                    nc.gpsimd.dma_gather(
                        m[:, :R_LO, :], table[:NLO], gi[:, :R_LO * 8],
                        R_LO * P, R_LO * P, elem, single_packet=False)
                    nc.gpsimd.dma_gather(
                        m[:, R_LO:, :], table[NLO:], gi[:, R_LO * 8:],
                        R_HI * P, R_HI * P, elem, single_packet=False)
                    if use_ew:
                        nc.vector.tensor_tensor(
                            out=m[:, :, k * H:(k + 1) * H],
                            in0=m[:, :, k * H:(k + 1) * H],
                            in1=ew[:, w, :, k // 2, k % 2, None]
                                .to_broadcast([P, RW, H]),
                            op=OP.mult)
                    s = build_S(w)
                    pa = psagg.tile([H, P], f32, tag="agg")
                    for r in range(RW):
                        nc.tensor.matmul(pa[:], m[:, r, k * H:k * H + H],
                                         s[:, r, :], start=(r == 0), stop=False)
                    nc.tensor.matmul(pa[:], selfW,
                                     src_strip[:, w * P:(w + 1) * P],
                                     start=False, stop=True)
                    nc.scalar.activation(dst_strip[:, w * P:(w + 1) * P], pa[:],
                                         AF.Copy)

            def ag8(in_t, out_t):
                nc.gpsimd.collective_compute(
                    "AllGather", OP.bypass,
                    replica_groups=[list(range(NCORES))],
                    ins=[in_t[:]], outs=[out_t[:]])

            # =================== ce GIN ===================
            strips = [xT, sA, sB, sA]
            for l in range(L):
                table = x_tab if l == 0 else tabs[(l + 1) % 2]
                aggregate(table, False, strips[l], W(f"ce_self_{l}"), aggT)
                mlp(strips[l + 1], aggT, W(f"ce_W1_{l}"), bias(f"ce_b1_{l}"),
                    W(f"ce_W2_{l}"), bias(f"ce_b2_{l}"))
                for w in range(WPC):
                    transpose_to(stagingP[:, w, :H],
                                 strips[l + 1][:, w * P:(w + 1) * P])
                nc.sync.dma_start(
                    payl[:].rearrange("(w p) e -> p w e", p=P),
                    stagingP[:, :, :2 * H])
                ag8(payl, tabZ if l == L - 1 else tabs[l % 2])
                if l == L - 1:
                    nc.sync.dma_start(tabZo[:], payl[:])
            ZT = strips[L]   # = sA

            # =================== h_orig partial ===================
            pho = psagg.tile([H, P], f32, tag="agg")
            for w in range(WPC):
                sb = wp2.tile([P, P], f16, tag="sbw")
                nc.vector.tensor_tensor(
                    out=sb[:], in0=brelo[:, w:w + 1].to_broadcast([P, P]),
                    in1=iota[:], op=OP.is_equal)
                nc.tensor.matmul(pho[:], stagingP[:, w, :H], sb[:],
                                 start=(w == 0), stop=(w == WPC - 1))
            ho_part = spool.tile([H, P], f32)
            nc.vector.tensor_copy(ho_part[:], pho[:])

            # =================== masks + masked_x ===================
            for (o, n) in CHUNKS:
                pnm = pstp.tile([P, 512], f32, tag="tp")
                for g in range(2):
                    k0, k1 = 2 * g, 2 * g + 1
                    p1 = psmlp.tile([P, 512], f32, tag="mlp")
                    nc.tensor.matmul(p1[:H, :n], W(f"nm_W1_{k0}"),
                                     ZT[:, o:o + n], start=True, stop=True)
                    nc.tensor.matmul(p1[H:, :n], W(f"nm_W1_{k1}"),
                                     ZT[:, o:o + n], start=True, stop=True)
                    h1 = wp3.tile([P, 512], f16, tag="mlph")
                    nc.scalar.activation(h1[:H, :n], p1[:H, :n], AF.Relu,
                                         bias=bias(f"nm_b1_{k0}"))
                    nc.scalar.activation(h1[H:, :n], p1[H:, :n], AF.Relu,
                                         bias=bias(f"nm_b1_{k1}"))
                    nc.tensor.matmul(pnm[32 * g:32 * g + 2, :n],
                                     Wq(f"nm_W2bd_p{g}"), h1[:, :n],
                                     start=True, stop=True)
                nmc = wp2.tile([34, 512], f16, tag="nmc")
                nc.scalar.activation(nmc[:, :n], pnm[:34, :n], AF.Sigmoid,
                                     bias=bias("nm_b2_q", 34))
                nc.sync.dma_start(nm_out[0:2, o:o + n], nmc[0:2, :n])
                nc.sync.dma_start(nm_out[2:4, o:o + n], nmc[32:34, :n])
                for k in range(K):
                    p2 = psmlp.tile([P, 512], f32, tag="mlp")
                    nc.tensor.matmul(p2[:H, :n], W(f"fm_W1_{k}"), ZT[:, o:o + n],
                                     start=True, stop=True)
                    h2 = wp3.tile([P, 512], f16, tag="mlph")
                    nc.scalar.activation(h2[:H, :n], p2[:H, :n], AF.Relu,
                                         bias=bias(f"fm_b1_{k}"))
                    p3 = psmlp.tile([P, 512], f32, tag="mlp")
                    nc.tensor.matmul(p3[:H, :n], W(f"fm_W2_{k}"), h2[:H, :n],
                                     start=True, stop=True)
                    fk = wp3.tile([P, 512], f16, tag="mlph")
                    nc.scalar.activation(fk[:H, :n], p3[:H, :n], AF.Sigmoid,
                                         bias=bias(f"fm_b2_{k}"))
                    nc.sync.dma_start(fm_out[k, :, o:o + n], fk[:H, :n])
                    pb = psmlp.tile([P, 512], f32, tag="mlp")
                    nc.tensor.matmul(pb[:H, :n], Wq(f"sel_{k}")[:34, :],
                                     nmc[:, :n], start=True, stop=True)
                    nmb = wp3.tile([P, 512], f16, tag="mlph")
                    nc.vector.tensor_copy(nmb[:H, :n], pb[:H, :n])
                    nc.vector.tensor_tensor(out=cldst[k][:, o:o + n],
                                            in0=xT[:, o:o + n],
                                            in1=fk[:H, :n], op=OP.mult)
                    nc.vector.tensor_tensor(out=cldst[k][:, o:o + n],
                                            in0=cldst[k][:, o:o + n],
                                            in1=nmb[:H, :n], op=OP.mult)
                for j in range(n // P):
                    w = (o + j * P) // P
                    for k in range(K):
                        transpose_to(stagingP[:, w, k * H:(k + 1) * H],
                                     cldst[k][:, w * P:(w + 1) * P])
            nc.sync.dma_start(
                paylP[:].rearrange("(w p) e -> p w e", p=P), stagingP[:])
            ag8(paylP, tabP[0])

            # =================== edge masks ===================
            # chunk plan over one window's slots: (src tile key, local off,
            # global off, len)
            emplan = []
            for ch in range(0, R_LO * P, 512):
                emplan.append(("L", ch, ch, min(512, R_LO * P - ch)))
            for ch in range(0, R_HI * P, 512):
                emplan.append(("Hh", ch, R_LO * P + ch, min(512, R_HI * P - ch)))
            for w in range(WPC):
                gi = load_idx(w, gidx_d)
                zsL = wp1.tile([P, 1, R_LO * P], f16, tag="zsL")
                zsH = wp1.tile([P, 1, R_HI * P], f16, tag="zsH")
                nc.gpsimd.dma_gather(zsL[:], tabZ[:NLO], gi[:, :R_LO * 8],
                                     R_LO * P, R_LO * P, 2 * H, transpose=True,
                                     single_packet=False)
                nc.gpsimd.dma_gather(zsH[:], tabZ[NLO:], gi[:, R_LO * 8:],
                                     R_HI * P, R_HI * P, 2 * H, transpose=True,
                                     single_packet=False)
                # dst side: Zdst[slot] = Z[window-node rel[slot]] via S^T
                # expansion from a sequential window load (no descriptors)
                znm = wp1.tile([P, 2 * H], f16, tag="znm")
                nc.sync.dma_start(znm[:], tabZo[w * P:(w + 1) * P, :])
                sw = build_S(w)
                zdT = wp1.tile([H, SLOTW], f16, tag="zdT")
                for r4 in range(0, RW, 4):
                    nr = min(4, RW - r4)
                    pz = psmlp.tile([P, 512], f32, tag="mlp")
                    for j in range(nr):
                        stp = pstp.tile([P, P], f16, tag="tp")
                        nc.tensor.transpose(out=stp[:], in_=sw[:, r4 + j, :],
                                            identity=ident[:])
                        stt = wp2.tile([P, P], f16, tag="stt")
                        nc.vector.tensor_copy(stt[:], stp[:])
                        nc.tensor.matmul(pz[:H, j * P:(j + 1) * P],
                                         znm[:, :H], stt[:],
                                         start=True, stop=True)
                    nc.scalar.activation(zdT[:, r4 * P:r4 * P + nr * P],
                                         pz[:H, :nr * P], AF.Copy)
                eo = wp1.tile([34, SLOTW], f16, tag="ewsig")
                for (skey, lo, go, n) in emplan:
                    zs = zsL if skey == "L" else zsH
                    po = psagg.tile([P, 512], f32, tag="agg")
                    for g in range(2):
                        ph2 = psmlp.tile([P, 512], f32, tag="mlp")
                        nc.tensor.matmul(ph2[:, :n], W(f"em_W1a_p{g}"),
                                         zs[:H, 0, lo:lo + n], start=True,
                                         stop=False)
                        nc.tensor.matmul(ph2[:, :n], W(f"em_W1b_p{g}"),
                                         zdT[:, go:go + n], start=False,
                                         stop=True)
                        hh = wp3.tile([P, 512], f16, tag="mlph")
                        if g == 0:
                            nc.scalar.activation(hh[:, :n], ph2[:, :n], AF.Relu,
                                                 bias=bias("em_b1_p0", P))
                        else:
                            nc.vector.tensor_scalar(
                                out=hh[:, :n], in0=ph2[:, :n],
                                scalar1=bias("em_b1_p1", P), scalar2=0.0,
                                op0=OP.add, op1=OP.max)
                        nc.tensor.matmul(po[32 * g:32 * g + 2, :n],
                                         Wq(f"em_W2bd_p{g}"), hh[:, :n],
                                         start=True, stop=True)
                    nc.scalar.activation(eo[:, go:go + n], po[:34, :n],
                                         AF.Sigmoid, bias=bias("em_b2_q", 34))
                # wrap [34, SLOTW] -> ew[:, w, r, g, q] via RW transposes
                pt = pstp.tile([P, RW * 34], f16, tag="tp")
                for r in range(RW):
                    nc.tensor.transpose(out=pt[:, 34 * r:34 * r + 34],
                                        in_=eo[:, r * P:(r + 1) * P],
                                        identity=ident[:34, :34])
                ptv = pt[:].rearrange("p (r e) -> p r e", r=RW)
                nc.vector.tensor_copy(ew[:, w, :, 0, :], ptv[:, :, 0:2])
                nc.vector.tensor_copy(ew[:, w, :, 1, :], ptv[:, :, 32:34])
            nc.sync.dma_start(ew_out[:], ew[:].rearrange("p w r g q -> p (w r g q)"))

            # =================== cl GIN ===================
            for l in range(L):
                table = tabP[l % 2]
                selfW = W(f"cl_self_{l}")
                for w in range(WPC):
                    gi = load_idx(w, gidx_d)
                    m = wp2.tile([P, RW, 4 * H], f16, tag="msg")
                    nc.gpsimd.dma_gather(
                        m[:, :R_LO, :], table[:NLO], gi[:, :R_LO * 8],
                        R_LO * P, R_LO * P, 4 * H, single_packet=False)
                    nc.gpsimd.dma_gather(
                        m[:, R_LO:, :], table[NLO:], gi[:, R_LO * 8:],
                        R_HI * P, R_HI * P, 4 * H, single_packet=False)
                    s = build_S(w)
                    for k in range(K):
                        nc.vector.tensor_tensor(
                            out=m[:, :, k * H:(k + 1) * H],
                            in0=m[:, :, k * H:(k + 1) * H],
                            in1=ew[:, w, :, k // 2, k % 2, None]
                                .to_broadcast([P, RW, H]),
                            op=OP.mult)
                        pa = psagg.tile([H, P], f32, tag="agg",
                                        name=f"clpa{k}")
                        for r in range(RW):
                            nc.tensor.matmul(pa[:], m[:, r, k * H:k * H + H],
                                             s[:, r, :], start=(r == 0),
                                             stop=False)
                        nc.tensor.matmul(pa[:], selfW,
                                         cldst[k][:, w * P:(w + 1) * P],
                                         start=False, stop=True)
                        nc.scalar.activation(
                            cldst[k][:, w * P:(w + 1) * P], pa[:], AF.Copy)
                for k in range(K):
                    mlp(cldst[k], cldst[k], W(f"cl_W1_{l}"), bias(f"cl_b1_{l}"),
                        W(f"cl_W2_{l}"), bias(f"cl_b2_{l}"))
                    for w in range(WPC):
                        transpose_to(stagingP[:, w, k * H:(k + 1) * H],
                                     cldst[k][:, w * P:(w + 1) * P])
                if l < L - 1:
                    nc.sync.dma_start(
                        paylP[:].rearrange("(w p) e -> p w e", p=P), stagingP[:])
                    ag8(paylP, tabP[(l + 1) % 2])

            # =================== pooling, h_stable, logits, h_orig ========
            pk = [psagg.tile([H, 256], f32, tag="agg", name=f"pk{k}") for k in range(K)]
            for w in range(WPC):
                sb2 = wp2.tile([P, 256], f16, tag="sb2")
                nc.vector.tensor_tensor(
                    out=sb2[:], in0=brel2[:, w:w + 1].to_broadcast([P, 256]),
                    in1=iota2[:], op=OP.is_equal)
                for k in range(K):
                    nc.tensor.matmul(pk[k][:], stagingP[:, w, k * H:(k + 1) * H],
                                     sb2[:], start=(w == 0), stop=(w == WPC - 1))
            for k in range(K):
                pp = wp2.tile([H, 256], f32, tag="poolp")
                nc.vector.tensor_copy(pp[:], pk[k][:])
                nc.sync.dma_start(
                    cc_in[k * H * B:(k + 1) * H * B]
                    .rearrange("(p e) -> p e", p=H), pp[:])
            nc.sync.dma_start(
                cc_in[K * H * B:].rearrange("(p e) -> p e", p=H), ho_part[:])
            ag8(cc_in, cc_out)

            # combine pools
            for k in range(K):
                acc = wp2.tile([H, 256], f32, tag="acc")
                tmp = wp1.tile([H, NCORES, 256], f32, tag="ptmp")
                nc.sync.dma_start(
                    tmp[:],
                    cc_out[:, k * H * B:(k + 1) * H * B]
                    .rearrange("c (p e) -> p c e", p=H))
                nc.vector.tensor_tensor(out=acc[:], in0=tmp[:, 0, :],
                                        in1=tmp[:, 1, :], op=OP.add)
                for j in range(2, NCORES):
                    nc.vector.tensor_tensor(out=acc[:], in0=acc[:],
                                            in1=tmp[:, j, :], op=OP.add)
                nc.vector.tensor_tensor(out=acc[:], in0=acc[:], in1=grecip[:],
                                        op=OP.mult)
                nc.sync.dma_start(hs_out[k], acc[:])
                h16 = wp1.tile([H, 256], f16, tag="h16")
                nc.vector.tensor_copy(h16[:], acc[:])
                pl = pstp.tile([16, 256], f32, tag="tp")
                nc.tensor.matmul(pl[:C, :], W(f"clf_W_{k}"), h16[:],
                                 start=True, stop=True)
                lgt = wp1.tile([16, 256], f32, tag="lgt")
                nc.vector.tensor_scalar(out=lgt[:C, :], in0=pl[:C, :],
                                        scalar1=bias(f"clf_b_{k}", C),
                                        scalar2=None, op0=OP.add)
                nc.sync.dma_start(lg_out[k, :C, :], lgt[:C, :])

            # combine h_orig
            hof = wp2.tile([H, 256], f32, tag="acc")
            nc.gpsimd.memset(hof[:], 0.0)
            hot = wp1.tile([H, NCORES, P], f32, tag="hot")
            nc.sync.dma_start(
                hot[:], cc_out[:, K * H * B:].rearrange("c (p e) -> p c e", p=H))
            for j in range(NCORES):
                g0 = gbase_core[j]
                nwid = min(P, B - g0)
                nc.vector.tensor_tensor(
                    out=hof[:, g0:g0 + nwid], in0=hof[:, g0:g0 + nwid],
                    in1=hot[:, j, :nwid], op=OP.add)
            nc.vector.tensor_tensor(out=hof[:], in0=hof[:], in1=grecip[:],
                                    op=OP.mult)
            nc.sync.dma_start(ho_out[:], hof[:])

    nc.compile()
    return nc


# ---------------------------------------------------------------- entry point
def make_in_maps(inputs, meta, per_core, post, wf16, wq16, bf32):
    x = np.asarray(inputs["x"], np.float32)
    xpad = np.zeros((NPAD, 2 * H), np.float16)
    xpad[:N, :H] = x.astype(np.float16)
    iota = np.tile(np.arange(P, dtype=np.float16), (P, 1))
    iota2 = np.tile(np.arange(256, dtype=np.float16), (P, 1))
    ident = np.eye(P, dtype=np.float16)
    ones64 = np.ones((1, H), np.float16)
    grecip64 = np.tile(post["grecip"].reshape(1, B), (H, 1)).astype(np.float32)

    in_maps = []
    for c in range(NCORES):
        d = per_core[c]
        xsl = np.zeros((H, NPC), np.float16)
        hi = min((c + 1) * NPC, N)
        xsl[:, : hi - c * NPC] = x[c * NPC:hi].T.astype(np.float16)
        in_maps.append({
            "x_tab": xpad, "xT_own": xsl,
            "gidx": d["gidx"], "grel": d["grel"], "didx": d["didx"],
            "brel_own": d["brel_own"], "brel256": d["brel256"],
            "iota": iota, "iota256": iota2, "ident": ident, "ones64": ones64,
            "wf16": wf16, "wq16": wq16, "bf32": bf32, "grecip64": grecip64,
        })
    return in_maps


def kernel(**inputs):
    x = np.asarray(inputs["x"], np.float32)
    meta, per_core, post = prep(x, inputs["edge_index"], inputs["batch"])
    wf16, wq16, bf32, wmap, qmap, bmap = pack_weights(inputs)
    meta["wmap"], meta["bmap"], meta["qmap"] = wmap, bmap, qmap
    meta["wc"], meta["bc"], meta["qc"] = (wf16.shape[1], bf32.shape[1],
                                          wq16.shape[1])
    nc = build_program(meta)
    in_maps = make_in_maps(inputs, meta, per_core, post, wf16, wq16, bf32)
    res = run_bass_kernel_spmd(nc, in_maps, core_ids=list(range(NCORES)))
    return assemble(res.results, meta, post)


def assemble(results, meta, post):
    RW = meta["RW"]
    # node-split outputs
    nm = np.concatenate([results[c]["nm_out"] for c in range(NCORES)], axis=1)
    fm = np.concatenate([results[c]["fm_out"] for c in range(NCORES)], axis=2)
    node_masks = nm.astype(np.float32).T[:N, :, None]            # [N, K, 1]
    feat_masks = fm.astype(np.float32).transpose(2, 0, 1)[:N]    # [N, K, F]

    # edge masks: ew_out [128, WPC*EMROWS]; value for (p, w, k, r) is slot
    # w*SLOTW + r*128 + p of that core.
    EMR = K * RW
    ews = np.stack([results[c]["ew_out"] for c in range(NCORES)])  # [8,128,WPC*EMR]
    ews = ews.reshape(NCORES, P, WPC, RW, K)
    soe = post["slot_of_edge"]
    c_e = soe // meta["TOT_PC"]
    rem = soe % meta["TOT_PC"]
    w_e = rem // meta["SLOTW"]
    r_e = (rem % meta["SLOTW"]) // P
    p_e = rem % P
    edge_masks = np.empty((E, K, 1), np.float32)
    for k in range(K):
        edge_masks[:, k, 0] = ews[c_e, p_e, w_e, r_e, k]

    hs = results[0]["hs_out"]          # [K, H, B]
    lg = results[0]["lg_out"][:, :C]   # [K, C, B]
    ho = results[0]["ho_out"]          # [H, B]
    expert_logits = lg.transpose(2, 0, 1).astype(np.float32)
    h_stable = hs.transpose(2, 0, 1).astype(np.float32)
    h_orig = ho.T.astype(np.float32)
    return (expert_logits, h_stable, h_orig, node_masks, edge_masks, feat_masks)
